# revision 13
# baseline (speedup 1.0000x reference)
"""Trainium2 Bass kernel for the AnaphoricityScorer (coref pairwise FFNN scorer).

Math (per batch row i, antecedent slot t):
    b  = all_mentions[top_indices[i, t]]                    # gathered mention
    pair = [a_i, b, a_i * b, pw[i, t]]                      # 3*1024 + 64 features
    h  = leaky_relu(pair @ W1.T + b1, 0.01)                 # 1024 hidden
    ffnn = h @ Wout.T + bout                                # scalar
    score = rough[i, t] + ffnn
    out = concat([eps_col, scores], axis=1)                 # [batch, 65]

Distribution: pure data parallel over the batch dim across 8 NeuronCores
(no collectives). all_mentions and FFNN weights are replicated.

Per-core algorithm (B = 128 batch rows -> 8192 pair rows, groups of 512):
  - The a-term a_i @ W1a.T is identical for all 64 antecedents of row i, so it
    is computed once per batch row in a prologue (ha = mentions @ W1a.T + b1)
    and broadcast-added into the pair-row PSUM with a stride-0 DVE add.
  - b rows arrive transposed (features on partitions) straight from HBM via
    dma_gather(transpose=True), which is exactly the matmul rhs layout.
  - a*b is built by a DVE multiply against a stride-0 broadcast of mentions^T.
  - One PSUM accumulation of 17 matmuls per (row-group, hidden-tile):
    8 K-tiles of W1b, 8 of W1ab, 1 of W1pw (K=64).
  - Lrelu on ScalarE evicts PSUM -> SBUF bf16; the Wout reduction is a
    K=128, M=1 matmul accumulated over the 8 hidden tiles.
  - bout + rough scores are added on DVE; one DMA out per core.

Everything is bf16 on the TensorEngine with fp32 PSUM accumulation.
"""

import sys

for _p in ("/opt/trn_rl_repo",):
    if _p not in sys.path:
        sys.path.append(_p)

import numpy as np
import ml_dtypes

import concourse.bacc as bacc
import concourse.mybir as mybir
from concourse.tile import TileContext
from concourse.bass_utils import run_bass_kernel_spmd

BF16 = mybir.dt.bfloat16
F32 = mybir.dt.float32
I16 = mybir.dt.int16
FP8 = mybir.dt.float8e4

USE_FP8 = True       # b/ab blocks in fp8-e4m3 DoubleRow (2 k-tiles per matmul)
FP8_SCALE = 512.0    # weight pre-scale so 0.02-magnitude weights leave fp8 denormals

N_CORES = 8
EMB = 1024
HID = 1024
N_ANTS = 64
PW = 64
EPS = 1e-7
GRP = 512          # pair rows per group (= 8 batch rows)
ROWS_PER_GRP = 8   # batch rows per group


def build_nc(B: int, n_tab: int):
    """Build the per-core Bass graph. B = batch rows per core."""
    G = (B * N_ANTS) // GRP  # number of row groups
    FC = EMB // 128          # 8 feature k-tiles per 1024-feature block
    NT = HID // 128          # 8 hidden tiles

    nc = bacc.Bacc("TRN2")
    amen = nc.declare_dram_parameter("amen", [n_tab, EMB], BF16, isOutput=False)
    amen8 = nc.declare_dram_parameter("amen8", [n_tab, EMB], FP8, isOutput=False)
    ment = nc.declare_dram_parameter("ment", [128, FC, B], BF16, isOutput=False)
    wdt = FP8 if USE_FP8 else BF16
    w1bt = nc.declare_dram_parameter("w1bt", [128, FC, HID], wdt, isOutput=False)
    w1abt = nc.declare_dram_parameter("w1abt", [128, FC, HID], wdt, isOutput=False)
    w1at = nc.declare_dram_parameter("w1at", [128, FC, HID], BF16, isOutput=False)
    w1pw = nc.declare_dram_parameter("w1pw", [128, HID], BF16, isOutput=False)
    woutt = nc.declare_dram_parameter("woutt", [128, NT], BF16, isOutput=False)
    pwt = nc.declare_dram_parameter("pwt", [128, B * N_ANTS], BF16, isOutput=False)
    idx = nc.declare_dram_parameter("idx", [128, G * (GRP // 16)], I16, isOutput=False)
    rough = nc.declare_dram_parameter("rough", [1, B * N_ANTS], F32, isOutput=False)
    out = nc.declare_dram_parameter("out", [B, N_ANTS], F32, isOutput=True)

    with TileContext(nc) as tc:
        with (
            tc.tile_pool(name="const", bufs=1) as const,
            tc.tile_pool(name="btp", bufs=5) as btp,
            tc.tile_pool(name="abtp", bufs=4) as abtp,
            tc.tile_pool(name="bt8p", bufs=4) as bt8p,
            tc.tile_pool(name="wgp", bufs=3) as wgp,
            tc.tile_pool(name="htp", bufs=10) as htp,
            tc.tile_pool(name="rpool", bufs=3) as rpool,
            tc.tile_pool(name="spool", bufs=2) as spool,
            tc.tile_pool(name="psum", bufs=3, space="PSUM") as psum_pool,
            tc.tile_pool(name="psum_s", bufs=2, space="PSUM") as psum_s_pool,
        ):
            # ---- resident loads (gather + prologue deps first) ------------
            idx_t = const.tile([128, G * (GRP // 16)], I16)
            nc.sync.dma_start(idx_t[:], idx[:, :])
            ment_t = const.tile([128, FC, B], BF16)
            nc.sync.dma_start(ment_t[:], ment[:, :, :])
            w1at_t = const.tile([128, FC, HID], BF16)
            nc.sync.dma_start(w1at_t[:], w1at[:, :, :])
            w1bt_t = const.tile([128, FC, HID], wdt)
            nc.sync.dma_start(w1bt_t[:], w1bt[:, :, :])
            w1abt_t = const.tile([128, FC, HID], wdt)
            nc.sync.dma_start(w1abt_t[:], w1abt[:, :, :])
            w1pw_t = const.tile([128, HID], BF16)
            nc.sync.dma_start(w1pw_t[:], w1pw[:, :])
            woutt_t = const.tile([128, NT], BF16)
            nc.sync.dma_start(woutt_t[:], woutt[:, :])
            pwt_t = const.tile([128, B * N_ANTS], BF16)
            nc.sync.dma_start(pwt_t[:], pwt[:, :])
            # ---- prologue: ha = mentions @ (W1a*S).T, rows-on-partitions --
            # ha2r regroups ha so group g's 8 batch rows sit on partitions
            # 64..71 of the per-group weight tile wg (spliced below); the
            # static pwt operand carries one-hot rows that select the batch
            # row, folding the a-term (and b1 via an all-ones row) into the
            # pw matmul for free.
            ha2 = const.tile([B, HID], BF16)
            for half in range(HID // 512):
                pp = psum_pool.tile([B, 512], F32)
                for fc in range(FC):
                    nc.tensor.matmul(
                        pp[:],
                        ment_t[:, fc, :],
                        w1at_t[:, fc, half * 512:(half + 1) * 512],
                        start=(fc == 0),
                        stop=(fc == FC - 1),
                    )
                nc.scalar.activation(
                    ha2[:, half * 512:(half + 1) * 512], pp[:],
                    mybir.ActivationFunctionType.Identity,
                )
            ha2_dram = nc.dram_tensor("ha2_scratch", [B, HID], BF16)
            nc.sync.dma_start(ha2_dram[:, :], ha2[:])
            ha2r = const.tile([8, G, HID], BF16)
            nc.sync.dma_start(
                ha2r[:],
                ha2_dram[:, :].rearrange("(g q) n -> q g n", q=ROWS_PER_GRP),
            )

            # ---- main loop over row groups --------------------------------
            # Software-pipelined emission: the gather + a*b multiplies for
            # group g+1 are emitted BEFORE group g's matmuls so the DVE
            # stream reaches them early, and each (g, nt) second-matmul is
            # deferred by one nt so its ht dependency never stalls PE.
            def produce_group(g):
                r0 = g * ROWS_PER_GRP
                rtile = rpool.tile([1, GRP], F32)
                nc.sync.dma_start(rtile[:], rough[0:1, g * GRP:(g + 1) * GRP])
                isl = idx_t[:, g * (GRP // 16):(g + 1) * (GRP // 16)]
                bt8 = bt8p.tile([128, FC, GRP], FP8)
                nc.gpsimd.dma_gather(bt8[:], amen8[:, :], isl, GRP, GRP, EMB,
                                     transpose=True)
                bt = btp.tile([128, FC, GRP], BF16)
                nc.gpsimd.dma_gather(bt[:], amen[:, :], isl, GRP, GRP, EMB,
                                     transpose=True)
                abt = abtp.tile([128, FC, GRP], FP8 if USE_FP8 else BF16)
                a_b = ment_t[:, :, r0:r0 + ROWS_PER_GRP]
                for fc in range(FC):
                    nc.vector.tensor_mul(
                        abt[:, fc, :].rearrange("p (a b) -> p a b", a=ROWS_PER_GRP),
                        bt[:, fc, :].rearrange("p (a b) -> p a b", a=ROWS_PER_GRP),
                        a_b[:, fc, :].unsqueeze(2).to_broadcast(
                            [128, ROWS_PER_GRP, N_ANTS]),
                    )
                if USE_FP8:
                    bt = bt8
                wg = wgp.tile([128, HID], BF16)
                nc.vector.tensor_copy(wg[:], w1pw_t[:])
                nc.vector.tensor_copy(wg[64:72, :], ha2r[:, g, :])
                return bt, abt, rtile, wg

            def emit_batch(ps4, hts, nts, start):
                # 4 M=1 matmuls packed into distinct PE column groups -- they
                # execute concurrently in the array (one per 32-col strip)
                for nt_i, ht_i in zip(nts, hts):
                    j = nt_i % 4
                    nc.tensor.matmul(
                        ps4[32 * j:32 * j + 1, :], woutt_t[:, nt_i:nt_i + 1],
                        ht_i[:], tile_position=(0, 32 * j),
                        start=start, stop=not start,
                    )

            def finalize_group(ps4, p_g, p_rtile):
                # DVE may read at most one PSUM operand per op: chain the four
                # column-group partial rows through SBUF
                t1 = spool.tile([1, GRP], F32)
                nc.vector.tensor_add(t1[:], ps4[0:1, :], p_rtile[:])
                t2 = spool.tile([1, GRP], F32)
                nc.vector.tensor_add(t2[:], ps4[32:33, :], t1[:])
                t3 = spool.tile([1, GRP], F32)
                nc.vector.tensor_add(t3[:], ps4[64:65, :], t2[:])
                stile = spool.tile([1, GRP], F32)
                nc.vector.tensor_add(stile[:], ps4[96:97, :], t3[:])
                nc.sync.dma_start(
                    out[p_g * ROWS_PER_GRP:(p_g + 1) * ROWS_PER_GRP, :].unsqueeze(0),
                    stile[:].rearrange("p (r c) -> p r c", r=ROWS_PER_GRP),
                )

            tiles = {0: produce_group(0)}
            rtiles = {}
            prev_group = None  # (g, ps4, hts) awaiting its second batch
            for g in range(G):
                r0 = g * ROWS_PER_GRP
                bt, abt, rtiles[g], wg = tiles.pop(g)
                if g + 1 < G:
                    tiles[g + 1] = produce_group(g + 1)
                hts = []
                ps4 = None
                for nt in range(NT):
                    ps = psum_pool.tile([128, GRP], F32)
                    nsl = slice(nt * 128, (nt + 1) * 128)
                    if USE_FP8:
                        for fc in range(0, FC, 2):
                            rhs = bt[:, fc:fc + 2, :].rearrange(
                                "p a b -> p (a b)").rearrange(
                                "p (i j) -> p j i", j=2)
                            nc.tensor.matmul(
                                ps[:], w1bt_t[:, fc:fc + 2, nsl], rhs,
                                perf_mode=mybir.MatmulPerfMode.DoubleRow,
                                start=(fc == 0), stop=False,
                            )
                        for fc in range(0, FC, 2):
                            nc.tensor.matmul(
                                ps[:], w1abt_t[:, fc:fc + 2, nsl], abt[:, fc:fc + 2, :],
                                perf_mode=mybir.MatmulPerfMode.DoubleRow,
                                start=False, stop=False,
                            )
                    else:
                        for fc in range(FC):
                            nc.tensor.matmul(
                                ps[:], w1bt_t[:, fc, nsl], bt[:, fc, :],
                                start=(fc == 0), stop=False,
                            )
                        for fc in range(FC):
                            nc.tensor.matmul(
                                ps[:], w1abt_t[:, fc, nsl], abt[:, fc, :],
                                start=False, stop=False,
                            )
                    nc.tensor.matmul(
                        ps[:], wg[:, nsl],
                        pwt_t[:, g * GRP:(g + 1) * GRP],
                        start=False, stop=True,
                    )
                    ht = htp.tile([128, GRP], BF16)
                    nc.scalar.activation(
                        ht[:], ps[:],
                        mybir.ActivationFunctionType.Lrelu, alpha=0.01,
                        scale=(1.0 / FP8_SCALE) if USE_FP8 else 1.0,
                    )
                    hts.append(ht)
                    if nt == 1 and prev_group is not None:
                        p_g, p_ps4, p_hts = prev_group
                        emit_batch(p_ps4, p_hts[4:8], range(4, 8), start=False)
                        finalize_group(p_ps4, p_g, rtiles.pop(p_g))
                        prev_group = None
                    if nt == 5:
                        ps4 = psum_s_pool.tile([128, GRP], F32)
                        emit_batch(ps4, hts[0:4], range(0, 4), start=True)
                prev_group = (g, ps4, hts)
            # flush the last group's second batch
            p_g, p_ps4, p_hts = prev_group
            emit_batch(p_ps4, p_hts[4:8], range(4, 8), start=False)
            finalize_group(p_ps4, p_g, rtiles.pop(p_g))

    nc.compile()
    return nc


def prep_inputs(all_mentions, mentions_batch, pw_batch, top_indices_batch,
                top_rough_scores_batch, W1, b1, Wout, bout, n_cores=N_CORES):
    """Host-side marshalling: shard over batch, cast/transpose into the
    layouts the kernel expects. Returns (in_maps, B, n_tab, bout_val)."""
    bf = ml_dtypes.bfloat16
    batch = mentions_batch.shape[0]
    B = batch // n_cores
    n_tab = all_mentions.shape[0]
    FC = EMB // 128
    NT = HID // 128
    G = (B * N_ANTS) // GRP

    amen = np.ascontiguousarray(all_mentions.astype(bf))
    amen8 = np.ascontiguousarray(
        np.clip(all_mentions, -240, 240).astype(ml_dtypes.float8_e4m3))

    def wt_block(Wcols, scale=1.0, dtype=bf):
        # [1024, 1024] f32 block -> [128, FC, HID] (feature on partitions)
        wt = Wcols.T.reshape(FC, 128, HID).transpose(1, 0, 2) * scale
        if dtype is not bf:
            wt = np.clip(wt, -240.0, 240.0)
        return np.ascontiguousarray(wt.astype(dtype))

    S = FP8_SCALE if USE_FP8 else 1.0
    f8 = ml_dtypes.float8_e4m3
    wdt = f8 if USE_FP8 else bf
    def wt_block_ilv(Wcols, scale, dtype):
        # fp8 transpose-gather pairing: dim1 index d=2c+j holds feature
        # f = 256c + 2p + j on partition p
        wt = Wcols.T.reshape(FC // 2, 128, 2, HID).transpose(1, 0, 2, 3) * scale
        wt = np.clip(wt, -240.0, 240.0).reshape(128, FC, HID)
        return np.ascontiguousarray(wt.astype(dtype))

    w1at = wt_block(W1[:, 0:EMB], S)
    w1bt = (wt_block_ilv(W1[:, EMB:2 * EMB], S, wdt) if USE_FP8
            else wt_block(W1[:, EMB:2 * EMB], S, wdt))
    w1abt = wt_block(W1[:, 2 * EMB:3 * EMB], S, wdt)
    w1pw = np.zeros((128, HID), dtype=bf)
    w1pw[:PW] = (W1[:, 3 * EMB:3 * EMB + PW].T * S).astype(bf)
    w1pw[72] = (b1 * S).astype(bf)
    woutt = np.ascontiguousarray(Wout[0].reshape(NT, 128).T.astype(bf))

    in_maps = []
    for c in range(n_cores):
        rows = slice(c * B, (c + 1) * B)
        m_c = np.asarray(mentions_batch[rows], dtype=np.float32)       # [B, 1024]
        ment = np.ascontiguousarray(
            m_c.T.reshape(FC, 128, B).transpose(1, 0, 2).astype(bf))   # [128, FC, B]
        pw_c = np.asarray(pw_batch[rows], dtype=np.float32)            # [B, 64, 64]
        pwt = np.zeros((128, B * N_ANTS), dtype=bf)
        pwt[:PW] = pw_c.reshape(B * N_ANTS, PW).T.astype(bf)
        cols = np.arange(B * N_ANTS)
        for q in range(ROWS_PER_GRP):
            pwt[PW + q] = ((cols % GRP) // N_ANTS == q).astype(bf)
        pwt[72] = np.ones(B * N_ANTS, dtype=bf)
        idx_c = np.asarray(top_indices_batch[rows]).astype(np.int64).reshape(-1)
        idx_tiles = []
        for g in range(G):
            v = idx_c[g * GRP:(g + 1) * GRP].astype(np.int16)
            idx_tiles.append(np.tile(v.reshape(GRP // 16, 16).T, (8, 1)))
        idx = np.ascontiguousarray(np.concatenate(idx_tiles, axis=1))  # [128, G*32]
        rough = np.ascontiguousarray(
            np.asarray(top_rough_scores_batch[rows], dtype=np.float32).reshape(1, -1)
            + np.float32(np.asarray(bout).reshape(-1)[0]))
        in_maps.append({
            "amen": amen, "amen8": amen8, "ment": ment, "w1bt": w1bt, "w1abt": w1abt,
            "w1at": w1at, "w1pw": w1pw, "woutt": woutt,
            "pwt": pwt, "idx": idx, "rough": rough,
        })
    return in_maps, B, n_tab


_NC_CACHE = {}


def kernel_with_results(all_mentions, mentions_batch, pw_batch, top_indices_batch,
                        top_rough_scores_batch, W1, b1, Wout, bout, **run_kwargs):
    in_maps, B, n_tab = prep_inputs(
        all_mentions, mentions_batch, pw_batch, top_indices_batch,
        top_rough_scores_batch, W1, b1, Wout, bout)
    key = (B, n_tab)
    if key not in _NC_CACHE:
        _NC_CACHE[key] = build_nc(B, n_tab)
    nc = _NC_CACHE[key]
    res = run_bass_kernel_spmd(nc, in_maps, list(range(N_CORES)), **run_kwargs)
    scores = np.concatenate([np.asarray(r["out"]) for r in res.results], axis=0)
    batch = scores.shape[0]
    full = np.empty((batch, N_ANTS + 1), np.float32)
    full[:, 0] = EPS
    full[:, 1:] = scores
    return full, res


def kernel(**inputs) -> np.ndarray:
    out, _ = kernel_with_results(**inputs)
    return out


# revision 14
# speedup vs baseline: 1.0370x; 1.0370x over previous
"""Trainium2 Bass kernel for the AnaphoricityScorer (coref pairwise FFNN scorer).

Math (per batch row i, antecedent slot t):
    b  = all_mentions[top_indices[i, t]]                    # gathered mention
    pair = [a_i, b, a_i * b, pw[i, t]]                      # 3*1024 + 64 features
    h  = leaky_relu(pair @ W1.T + b1, 0.01)                 # 1024 hidden
    ffnn = h @ Wout.T + bout                                # scalar
    score = rough[i, t] + ffnn
    out = concat([eps_col, scores], axis=1)                 # [batch, 65]

Distribution: pure data parallel over the batch dim across 8 NeuronCores
(no collectives). all_mentions and FFNN weights are replicated.

Per-core algorithm (B = 128 batch rows -> 8192 pair rows, groups of 512):
  - The a-term a_i @ W1a.T is identical for all 64 antecedents of row i, so it
    is computed once per batch row in a prologue (ha = mentions @ W1a.T + b1)
    and broadcast-added into the pair-row PSUM with a stride-0 DVE add.
  - b rows arrive transposed (features on partitions) straight from HBM via
    dma_gather(transpose=True), which is exactly the matmul rhs layout.
  - a*b is built by a DVE multiply against a stride-0 broadcast of mentions^T.
  - One PSUM accumulation of 17 matmuls per (row-group, hidden-tile):
    8 K-tiles of W1b, 8 of W1ab, 1 of W1pw (K=64).
  - Lrelu on ScalarE evicts PSUM -> SBUF bf16; the Wout reduction is a
    K=128, M=1 matmul accumulated over the 8 hidden tiles.
  - bout + rough scores are added on DVE; one DMA out per core.

Everything is bf16 on the TensorEngine with fp32 PSUM accumulation.
"""

import sys

for _p in ("/opt/trn_rl_repo",):
    if _p not in sys.path:
        sys.path.append(_p)

import numpy as np
import ml_dtypes

import concourse.bacc as bacc
import concourse.mybir as mybir
from concourse.tile import TileContext
from concourse.bass_utils import run_bass_kernel_spmd

BF16 = mybir.dt.bfloat16
F32 = mybir.dt.float32
I16 = mybir.dt.int16
FP8 = mybir.dt.float8e4

USE_FP8 = True       # b/ab blocks in fp8-e4m3 DoubleRow (2 k-tiles per matmul)
FP8_SCALE = 512.0    # weight pre-scale so 0.02-magnitude weights leave fp8 denormals

N_CORES = 8
EMB = 1024
HID = 1024
N_ANTS = 64
PW = 64
EPS = 1e-7
GRP = 512          # pair rows per group (= 8 batch rows)
ROWS_PER_GRP = 8   # batch rows per group


def build_nc(B: int, n_tab: int):
    """Build the per-core Bass graph. B = batch rows per core."""
    G = (B * N_ANTS) // GRP  # number of row groups
    FC = EMB // 128          # 8 feature k-tiles per 1024-feature block
    NT = HID // 128          # 8 hidden tiles

    nc = bacc.Bacc("TRN2")
    amen = nc.declare_dram_parameter("amen", [n_tab, EMB], BF16, isOutput=False)
    ment = nc.declare_dram_parameter("ment", [128, FC, B], BF16, isOutput=False)
    wdt = FP8 if USE_FP8 else BF16
    w1bt = nc.declare_dram_parameter("w1bt", [128, FC, HID], wdt, isOutput=False)
    w1abt = nc.declare_dram_parameter("w1abt", [128, FC, HID], wdt, isOutput=False)
    w1at = nc.declare_dram_parameter("w1at", [128, FC, HID], BF16, isOutput=False)
    w1pw = nc.declare_dram_parameter("w1pw", [128, HID], BF16, isOutput=False)
    woutt = nc.declare_dram_parameter("woutt", [128, NT], BF16, isOutput=False)
    pwt = nc.declare_dram_parameter("pwt", [128, B * N_ANTS], BF16, isOutput=False)
    idx = nc.declare_dram_parameter("idx", [128, G * (GRP // 16)], I16, isOutput=False)
    rough = nc.declare_dram_parameter("rough", [1, B * N_ANTS], F32, isOutput=False)
    out = nc.declare_dram_parameter("out", [B, N_ANTS], F32, isOutput=True)

    with TileContext(nc) as tc:
        with (
            tc.tile_pool(name="const", bufs=1) as const,
            tc.tile_pool(name="btp", bufs=5) as btp,
            tc.tile_pool(name="abtp", bufs=4) as abtp,
            tc.tile_pool(name="bt8p", bufs=4) as bt8p,
            tc.tile_pool(name="wgp", bufs=3) as wgp,
            tc.tile_pool(name="htp", bufs=10) as htp,
            tc.tile_pool(name="rpool", bufs=3) as rpool,
            tc.tile_pool(name="spool", bufs=2) as spool,
            tc.tile_pool(name="psum", bufs=3, space="PSUM") as psum_pool,
            tc.tile_pool(name="psum_s", bufs=2, space="PSUM") as psum_s_pool,
        ):
            # ---- resident loads (gather + prologue deps first) ------------
            idx_t = const.tile([128, G * (GRP // 16)], I16)
            nc.sync.dma_start(idx_t[:], idx[:, :])
            ment_t = const.tile([128, FC, B], BF16)
            nc.sync.dma_start(ment_t[:], ment[:, :, :])
            w1at_t = const.tile([128, FC, HID], BF16)
            nc.sync.dma_start(w1at_t[:], w1at[:, :, :])
            w1bt_t = const.tile([128, FC, HID], wdt)
            nc.sync.dma_start(w1bt_t[:], w1bt[:, :, :])
            w1abt_t = const.tile([128, FC, HID], wdt)
            nc.sync.dma_start(w1abt_t[:], w1abt[:, :, :])
            w1pw_t = const.tile([128, HID], BF16)
            nc.sync.dma_start(w1pw_t[:], w1pw[:, :])
            woutt_t = const.tile([128, NT], BF16)
            nc.sync.dma_start(woutt_t[:], woutt[:, :])
            pwt_t = const.tile([128, B * N_ANTS], BF16)
            nc.sync.dma_start(pwt_t[:], pwt[:, :])
            # ---- prologue: ha = mentions @ (W1a*S).T, rows-on-partitions --
            # ha2r regroups ha so group g's 8 batch rows sit on partitions
            # 64..71 of the per-group weight tile wg (spliced below); the
            # static pwt operand carries one-hot rows that select the batch
            # row, folding the a-term (and b1 via an all-ones row) into the
            # pw matmul for free.
            ha2 = const.tile([B, HID], BF16)
            for half in range(HID // 512):
                pp = psum_pool.tile([B, 512], F32)
                for fc in range(FC):
                    nc.tensor.matmul(
                        pp[:],
                        ment_t[:, fc, :],
                        w1at_t[:, fc, half * 512:(half + 1) * 512],
                        start=(fc == 0),
                        stop=(fc == FC - 1),
                    )
                nc.scalar.activation(
                    ha2[:, half * 512:(half + 1) * 512], pp[:],
                    mybir.ActivationFunctionType.Identity,
                )
            ha2_dram = nc.dram_tensor("ha2_scratch", [B, HID], BF16)
            nc.sync.dma_start(ha2_dram[:, :], ha2[:])
            ha2r = const.tile([8, G, HID], BF16)
            nc.sync.dma_start(
                ha2r[:],
                ha2_dram[:, :].rearrange("(g q) n -> q g n", q=ROWS_PER_GRP),
            )

            # ---- main loop over row groups --------------------------------
            # Software-pipelined emission: the gather + a*b multiplies for
            # group g+1 are emitted BEFORE group g's matmuls so the DVE
            # stream reaches them early, and each (g, nt) second-matmul is
            # deferred by one nt so its ht dependency never stalls PE.
            def produce_group(g):
                r0 = g * ROWS_PER_GRP
                rtile = rpool.tile([1, GRP], F32)
                nc.sync.dma_start(rtile[:], rough[0:1, g * GRP:(g + 1) * GRP])
                bt = btp.tile([128, FC, GRP], BF16)
                nc.gpsimd.dma_gather(
                    bt[:], amen[:, :],
                    idx_t[:, g * (GRP // 16):(g + 1) * (GRP // 16)],
                    GRP, GRP, EMB, transpose=True,
                )
                abt = abtp.tile([128, FC, GRP], FP8 if USE_FP8 else BF16)
                a_b = ment_t[:, :, r0:r0 + ROWS_PER_GRP]
                for fc in range(FC):
                    nc.vector.tensor_mul(
                        abt[:, fc, :].rearrange("p (a b) -> p a b", a=ROWS_PER_GRP),
                        bt[:, fc, :].rearrange("p (a b) -> p a b", a=ROWS_PER_GRP),
                        a_b[:, fc, :].unsqueeze(2).to_broadcast(
                            [128, ROWS_PER_GRP, N_ANTS]),
                    )
                if USE_FP8:
                    bt8 = bt8p.tile([128, FC, GRP], FP8)
                    for fc in range(FC):
                        nc.scalar.activation(
                            bt8[:, fc, :], bt[:, fc, :],
                            mybir.ActivationFunctionType.Identity)
                    bt = bt8
                wg = wgp.tile([128, HID], BF16)
                nc.vector.tensor_copy(wg[:], w1pw_t[:])
                nc.vector.tensor_copy(wg[64:72, :], ha2r[:, g, :])
                return bt, abt, rtile, wg

            def emit_batch(ps4, hts, nts, start):
                # 4 M=1 matmuls packed into distinct PE column groups -- they
                # execute concurrently in the array (one per 32-col strip)
                for nt_i, ht_i in zip(nts, hts):
                    j = nt_i % 4
                    nc.tensor.matmul(
                        ps4[32 * j:32 * j + 1, :], woutt_t[:, nt_i:nt_i + 1],
                        ht_i[:], tile_position=(0, 32 * j),
                        start=start, stop=not start,
                    )

            def finalize_group(ps4, p_g, p_rtile):
                # DVE may read at most one PSUM operand per op: chain the four
                # column-group partial rows through SBUF
                t1 = spool.tile([1, GRP], F32)
                nc.vector.tensor_add(t1[:], ps4[0:1, :], p_rtile[:])
                t2 = spool.tile([1, GRP], F32)
                nc.vector.tensor_add(t2[:], ps4[32:33, :], t1[:])
                t3 = spool.tile([1, GRP], F32)
                nc.vector.tensor_add(t3[:], ps4[64:65, :], t2[:])
                stile = spool.tile([1, GRP], F32)
                nc.vector.tensor_add(stile[:], ps4[96:97, :], t3[:])
                nc.sync.dma_start(
                    out[p_g * ROWS_PER_GRP:(p_g + 1) * ROWS_PER_GRP, :].unsqueeze(0),
                    stile[:].rearrange("p (r c) -> p r c", r=ROWS_PER_GRP),
                )

            tiles = {0: produce_group(0)}
            rtiles = {}
            prev_group = None  # (g, ps4, hts) awaiting its second batch
            for g in range(G):
                r0 = g * ROWS_PER_GRP
                bt, abt, rtiles[g], wg = tiles.pop(g)
                if g + 1 < G:
                    tiles[g + 1] = produce_group(g + 1)
                hts = []
                ps4 = None
                for nt in range(NT):
                    ps = psum_pool.tile([128, GRP], F32)
                    nsl = slice(nt * 128, (nt + 1) * 128)
                    if USE_FP8:
                        for fc in range(0, FC, 2):
                            nc.tensor.matmul(
                                ps[:], w1bt_t[:, fc:fc + 2, nsl], bt[:, fc:fc + 2, :],
                                perf_mode=mybir.MatmulPerfMode.DoubleRow,
                                start=(fc == 0), stop=False,
                            )
                        for fc in range(0, FC, 2):
                            nc.tensor.matmul(
                                ps[:], w1abt_t[:, fc:fc + 2, nsl], abt[:, fc:fc + 2, :],
                                perf_mode=mybir.MatmulPerfMode.DoubleRow,
                                start=False, stop=False,
                            )
                    else:
                        for fc in range(FC):
                            nc.tensor.matmul(
                                ps[:], w1bt_t[:, fc, nsl], bt[:, fc, :],
                                start=(fc == 0), stop=False,
                            )
                        for fc in range(FC):
                            nc.tensor.matmul(
                                ps[:], w1abt_t[:, fc, nsl], abt[:, fc, :],
                                start=False, stop=False,
                            )
                    nc.tensor.matmul(
                        ps[:], wg[:, nsl],
                        pwt_t[:, g * GRP:(g + 1) * GRP],
                        start=False, stop=True,
                    )
                    ht = htp.tile([128, GRP], BF16)
                    nc.scalar.activation(
                        ht[:], ps[:],
                        mybir.ActivationFunctionType.Lrelu, alpha=0.01,
                        scale=(1.0 / FP8_SCALE) if USE_FP8 else 1.0,
                    )
                    hts.append(ht)
                    if nt == 1 and prev_group is not None:
                        p_g, p_ps4, p_hts = prev_group
                        emit_batch(p_ps4, p_hts[4:8], range(4, 8), start=False)
                        finalize_group(p_ps4, p_g, rtiles.pop(p_g))
                        prev_group = None
                    if nt == 5:
                        ps4 = psum_s_pool.tile([128, GRP], F32)
                        emit_batch(ps4, hts[0:4], range(0, 4), start=True)
                prev_group = (g, ps4, hts)
            # flush the last group's second batch
            p_g, p_ps4, p_hts = prev_group
            emit_batch(p_ps4, p_hts[4:8], range(4, 8), start=False)
            finalize_group(p_ps4, p_g, rtiles.pop(p_g))

    nc.compile()
    return nc


def prep_inputs(all_mentions, mentions_batch, pw_batch, top_indices_batch,
                top_rough_scores_batch, W1, b1, Wout, bout, n_cores=N_CORES):
    """Host-side marshalling: shard over batch, cast/transpose into the
    layouts the kernel expects. Returns (in_maps, B, n_tab, bout_val)."""
    bf = ml_dtypes.bfloat16
    batch = mentions_batch.shape[0]
    B = batch // n_cores
    n_tab = all_mentions.shape[0]
    FC = EMB // 128
    NT = HID // 128
    G = (B * N_ANTS) // GRP

    amen = np.ascontiguousarray(all_mentions.astype(bf))

    def wt_block(Wcols, scale=1.0, dtype=bf):
        # [1024, 1024] f32 block -> [128, FC, HID] (feature on partitions)
        wt = Wcols.T.reshape(FC, 128, HID).transpose(1, 0, 2) * scale
        if dtype is not bf:
            wt = np.clip(wt, -240.0, 240.0)
        return np.ascontiguousarray(wt.astype(dtype))

    S = FP8_SCALE if USE_FP8 else 1.0
    f8 = ml_dtypes.float8_e4m3
    wdt = f8 if USE_FP8 else bf
    w1at = wt_block(W1[:, 0:EMB], S)
    w1bt = wt_block(W1[:, EMB:2 * EMB], S, wdt)
    w1abt = wt_block(W1[:, 2 * EMB:3 * EMB], S, wdt)
    w1pw = np.zeros((128, HID), dtype=bf)
    w1pw[:PW] = (W1[:, 3 * EMB:3 * EMB + PW].T * S).astype(bf)
    w1pw[72] = (b1 * S).astype(bf)
    woutt = np.ascontiguousarray(Wout[0].reshape(NT, 128).T.astype(bf))

    in_maps = []
    for c in range(n_cores):
        rows = slice(c * B, (c + 1) * B)
        m_c = np.asarray(mentions_batch[rows], dtype=np.float32)       # [B, 1024]
        ment = np.ascontiguousarray(
            m_c.T.reshape(FC, 128, B).transpose(1, 0, 2).astype(bf))   # [128, FC, B]
        pw_c = np.asarray(pw_batch[rows], dtype=np.float32)            # [B, 64, 64]
        pwt = np.zeros((128, B * N_ANTS), dtype=bf)
        pwt[:PW] = pw_c.reshape(B * N_ANTS, PW).T.astype(bf)
        cols = np.arange(B * N_ANTS)
        for q in range(ROWS_PER_GRP):
            pwt[PW + q] = ((cols % GRP) // N_ANTS == q).astype(bf)
        pwt[72] = np.ones(B * N_ANTS, dtype=bf)
        idx_c = np.asarray(top_indices_batch[rows]).astype(np.int64).reshape(-1)
        idx_tiles = []
        for g in range(G):
            v = idx_c[g * GRP:(g + 1) * GRP].astype(np.int16)
            idx_tiles.append(np.tile(v.reshape(GRP // 16, 16).T, (8, 1)))
        idx = np.ascontiguousarray(np.concatenate(idx_tiles, axis=1))  # [128, G*32]
        rough = np.ascontiguousarray(
            np.asarray(top_rough_scores_batch[rows], dtype=np.float32).reshape(1, -1)
            + np.float32(np.asarray(bout).reshape(-1)[0]))
        in_maps.append({
            "amen": amen, "ment": ment, "w1bt": w1bt, "w1abt": w1abt,
            "w1at": w1at, "w1pw": w1pw, "woutt": woutt,
            "pwt": pwt, "idx": idx, "rough": rough,
        })
    return in_maps, B, n_tab


_NC_CACHE = {}


def kernel_with_results(all_mentions, mentions_batch, pw_batch, top_indices_batch,
                        top_rough_scores_batch, W1, b1, Wout, bout, **run_kwargs):
    in_maps, B, n_tab = prep_inputs(
        all_mentions, mentions_batch, pw_batch, top_indices_batch,
        top_rough_scores_batch, W1, b1, Wout, bout)
    key = (B, n_tab)
    if key not in _NC_CACHE:
        _NC_CACHE[key] = build_nc(B, n_tab)
    nc = _NC_CACHE[key]
    res = run_bass_kernel_spmd(nc, in_maps, list(range(N_CORES)), **run_kwargs)
    scores = np.concatenate([np.asarray(r["out"]) for r in res.results], axis=0)
    batch = scores.shape[0]
    full = np.empty((batch, N_ANTS + 1), np.float32)
    full[:, 0] = EPS
    full[:, 1:] = scores
    return full, res


def kernel(**inputs) -> np.ndarray:
    out, _ = kernel_with_results(**inputs)
    return out


# revision 15
# speedup vs baseline: 1.0583x; 1.0205x over previous
"""Trainium2 Bass kernel for the AnaphoricityScorer (coref pairwise FFNN scorer).

Math (per batch row i, antecedent slot t):
    b  = all_mentions[top_indices[i, t]]                    # gathered mention
    pair = [a_i, b, a_i * b, pw[i, t]]                      # 3*1024 + 64 features
    h  = leaky_relu(pair @ W1.T + b1, 0.01)                 # 1024 hidden
    ffnn = h @ Wout.T + bout                                # scalar
    score = rough[i, t] + ffnn
    out = concat([eps_col, scores], axis=1)                 # [batch, 65]

Distribution: pure data parallel over the batch dim across 8 NeuronCores
(no collectives). all_mentions and FFNN weights are replicated.

Per-core algorithm (B = 128 batch rows -> 8192 pair rows, groups of 512):
  - The a-term a_i @ W1a.T is identical for all 64 antecedents of row i, so it
    is computed once per batch row in a prologue (ha = mentions @ W1a.T + b1)
    and broadcast-added into the pair-row PSUM with a stride-0 DVE add.
  - b rows arrive transposed (features on partitions) straight from HBM via
    dma_gather(transpose=True), which is exactly the matmul rhs layout.
  - a*b is built by a DVE multiply against a stride-0 broadcast of mentions^T.
  - One PSUM accumulation of 17 matmuls per (row-group, hidden-tile):
    8 K-tiles of W1b, 8 of W1ab, 1 of W1pw (K=64).
  - Lrelu on ScalarE evicts PSUM -> SBUF bf16; the Wout reduction is a
    K=128, M=1 matmul accumulated over the 8 hidden tiles.
  - bout + rough scores are added on DVE; one DMA out per core.

Everything is bf16 on the TensorEngine with fp32 PSUM accumulation.
"""

import sys

for _p in ("/opt/trn_rl_repo",):
    if _p not in sys.path:
        sys.path.append(_p)

import numpy as np
import ml_dtypes

import concourse.bacc as bacc
import concourse.mybir as mybir
from concourse.tile import TileContext
from concourse.bass_utils import run_bass_kernel_spmd

BF16 = mybir.dt.bfloat16
F32 = mybir.dt.float32
I16 = mybir.dt.int16
FP8 = mybir.dt.float8e4

USE_FP8 = True       # b/ab blocks in fp8-e4m3 DoubleRow (2 k-tiles per matmul)
FP8_SCALE = 512.0    # weight pre-scale so 0.02-magnitude weights leave fp8 denormals

N_CORES = 8
EMB = 1024
HID = 1024
N_ANTS = 64
PW = 64
EPS = 1e-7
GRP = 512          # pair rows per group (= 8 batch rows)
ROWS_PER_GRP = 8   # batch rows per group


def build_nc(B: int, n_tab: int):
    """Build the per-core Bass graph. B = batch rows per core."""
    G = (B * N_ANTS) // GRP  # number of row groups
    FC = EMB // 128          # 8 feature k-tiles per 1024-feature block
    NT = HID // 128          # 8 hidden tiles

    nc = bacc.Bacc("TRN2")
    amen = nc.declare_dram_parameter("amen", [n_tab, EMB], BF16, isOutput=False)
    ment = nc.declare_dram_parameter("ment", [128, FC, B], BF16, isOutput=False)
    wdt = FP8 if USE_FP8 else BF16
    w1bt = nc.declare_dram_parameter("w1bt", [128, FC, HID], wdt, isOutput=False)
    w1abt = nc.declare_dram_parameter("w1abt", [128, FC, HID], wdt, isOutput=False)
    w1at = nc.declare_dram_parameter("w1at", [128, FC, HID], BF16, isOutput=False)
    w1pw = nc.declare_dram_parameter("w1pw", [128, HID], BF16, isOutput=False)
    woutt = nc.declare_dram_parameter("woutt", [128, NT], BF16, isOutput=False)
    pwt = nc.declare_dram_parameter("pwt", [128, B * N_ANTS], BF16, isOutput=False)
    idx = nc.declare_dram_parameter("idx", [128, G * (GRP // 16)], I16, isOutput=False)
    rough = nc.declare_dram_parameter("rough", [1, B * N_ANTS], F32, isOutput=False)
    out = nc.declare_dram_parameter("out", [B, N_ANTS], F32, isOutput=True)

    with TileContext(nc) as tc:
        with (
            tc.tile_pool(name="const", bufs=1) as const,
            tc.tile_pool(name="btp", bufs=5) as btp,
            tc.tile_pool(name="abtp", bufs=4) as abtp,
            tc.tile_pool(name="bt8p", bufs=4) as bt8p,
            tc.tile_pool(name="wgp", bufs=3) as wgp,
            tc.tile_pool(name="htp", bufs=10) as htp,
            tc.tile_pool(name="rpool", bufs=3) as rpool,
            tc.tile_pool(name="spool", bufs=2) as spool,
            tc.tile_pool(name="psum", bufs=3, space="PSUM") as psum_pool,
            tc.tile_pool(name="psum_s", bufs=2, space="PSUM") as psum_s_pool,
        ):
            # ---- resident loads (gather + prologue deps first) ------------
            idx_t = const.tile([128, G * (GRP // 16)], I16)
            nc.sync.dma_start(idx_t[:], idx[:, :])
            ment_t = const.tile([128, FC, B], BF16)
            nc.sync.dma_start(ment_t[:], ment[:, :, :])
            w1at_t = const.tile([128, FC, HID], BF16)
            nc.sync.dma_start(w1at_t[:], w1at[:, :, :])
            w1bt_t = const.tile([128, FC, HID], wdt)
            nc.sync.dma_start(w1bt_t[:], w1bt[:, :, :])
            w1abt_t = const.tile([128, FC, HID], wdt)
            nc.sync.dma_start(w1abt_t[:], w1abt[:, :, :])
            w1pw_t = const.tile([128, HID], BF16)
            nc.sync.dma_start(w1pw_t[:], w1pw[:, :])
            woutt_t = const.tile([128, NT], BF16)
            nc.sync.dma_start(woutt_t[:], woutt[:, :])
            pwt_t = const.tile([128, B * N_ANTS], BF16)
            nc.sync.dma_start(pwt_t[:], pwt[:, :])
            # ---- prologue: ha = mentions @ (W1a*S).T, rows-on-partitions --
            # ha2r regroups ha so group g's 8 batch rows sit on partitions
            # 64..71 of the per-group weight tile wg (spliced below); the
            # static pwt operand carries one-hot rows that select the batch
            # row, folding the a-term (and b1 via an all-ones row) into the
            # pw matmul for free.
            ha2 = const.tile([B, HID], BF16)
            for half in range(HID // 512):
                pp = psum_pool.tile([B, 512], F32)
                for fc in range(FC):
                    nc.tensor.matmul(
                        pp[:],
                        ment_t[:, fc, :],
                        w1at_t[:, fc, half * 512:(half + 1) * 512],
                        start=(fc == 0),
                        stop=(fc == FC - 1),
                    )
                nc.scalar.activation(
                    ha2[:, half * 512:(half + 1) * 512], pp[:],
                    mybir.ActivationFunctionType.Identity,
                )
            ha2_dram = nc.dram_tensor("ha2_scratch", [B, HID], BF16)
            nc.sync.dma_start(ha2_dram[:, :], ha2[:])
            ha2r = const.tile([8, G, HID], BF16)
            nc.sync.dma_start(
                ha2r[:],
                ha2_dram[:, :].rearrange("(g q) n -> q g n", q=ROWS_PER_GRP),
            )

            # ---- main loop over row groups --------------------------------
            # Software-pipelined emission: the gather + a*b multiplies for
            # group g+1 are emitted BEFORE group g's matmuls so the DVE
            # stream reaches them early, and each (g, nt) second-matmul is
            # deferred by one nt so its ht dependency never stalls PE.
            def produce_group(g):
                r0 = g * ROWS_PER_GRP
                rtile = rpool.tile([1, GRP], F32)
                nc.sync.dma_start(rtile[:], rough[0:1, g * GRP:(g + 1) * GRP])
                bt = btp.tile([128, FC, GRP], BF16)
                nc.gpsimd.dma_gather(
                    bt[:], amen[:, :],
                    idx_t[:, g * (GRP // 16):(g + 1) * (GRP // 16)],
                    GRP, GRP, EMB, transpose=True,
                )
                abt = abtp.tile([128, FC, GRP], FP8 if USE_FP8 else BF16)
                a_b = ment_t[:, :, r0:r0 + ROWS_PER_GRP]
                for fc in range(FC):
                    nc.vector.tensor_mul(
                        abt[:, fc, :].rearrange("p (a b) -> p a b", a=ROWS_PER_GRP),
                        bt[:, fc, :].rearrange("p (a b) -> p a b", a=ROWS_PER_GRP),
                        a_b[:, fc, :].unsqueeze(2).to_broadcast(
                            [128, ROWS_PER_GRP, N_ANTS]),
                    )
                if USE_FP8:
                    bt8 = bt8p.tile([128, FC, GRP], FP8)
                    for fc in range(FC):
                        nc.scalar.activation(
                            bt8[:, fc, :], bt[:, fc, :],
                            mybir.ActivationFunctionType.Identity)
                    bt = bt8
                wg = wgp.tile([128, HID], BF16)
                nc.vector.tensor_copy(wg[:], w1pw_t[:])
                nc.vector.tensor_copy(wg[64:72, :], ha2r[:, g, :])
                return bt, abt, rtile, wg

            def emit_batch(ps4, hts, nts, start):
                # 4 M=1 matmuls packed into distinct PE column groups -- they
                # execute concurrently in the array (one per 32-col strip)
                for nt_i, ht_i in zip(nts, hts):
                    j = nt_i % 4
                    nc.tensor.matmul(
                        ps4[32 * j:32 * j + 1, :], woutt_t[:, nt_i:nt_i + 1],
                        ht_i[:], tile_position=(0, 32 * j),
                        start=start, stop=not start,
                    )

            def finalize_group(ps4, p_g, p_rtile):
                # DVE may read at most one PSUM operand per op: chain the four
                # column-group partial rows through SBUF
                t1 = spool.tile([1, GRP], F32)
                nc.vector.tensor_add(t1[:], ps4[0:1, :], p_rtile[:])
                t2 = spool.tile([1, GRP], F32)
                nc.vector.tensor_add(t2[:], ps4[32:33, :], t1[:])
                t3 = spool.tile([1, GRP], F32)
                nc.vector.tensor_add(t3[:], ps4[64:65, :], t2[:])
                stile = spool.tile([1, GRP], F32)
                nc.vector.tensor_add(stile[:], ps4[96:97, :], t3[:])
                nc.sync.dma_start(
                    out[p_g * ROWS_PER_GRP:(p_g + 1) * ROWS_PER_GRP, :].unsqueeze(0),
                    stile[:].rearrange("p (r c) -> p r c", r=ROWS_PER_GRP),
                )

            tiles = {0: produce_group(0)}
            rtiles = {}
            prev_group = None  # (g, ps4, hts) awaiting its second batch
            for g in range(G):
                r0 = g * ROWS_PER_GRP
                bt, abt, rtiles[g], wg = tiles.pop(g)
                if g + 1 < G:
                    tiles[g + 1] = produce_group(g + 1)
                hts = []
                ps4 = None
                for nt in range(NT):
                    ps = psum_pool.tile([128, GRP], F32)
                    nsl = slice(nt * 128, (nt + 1) * 128)
                    if USE_FP8:
                        for fc in range(0, FC, 2):
                            nc.tensor.matmul(
                                ps[:], w1bt_t[:, fc:fc + 2, nsl], bt[:, fc:fc + 2, :],
                                perf_mode=mybir.MatmulPerfMode.DoubleRow,
                                start=(fc == 0), stop=False,
                            )
                        for fc in range(0, FC, 2):
                            nc.tensor.matmul(
                                ps[:], w1abt_t[:, fc:fc + 2, nsl], abt[:, fc:fc + 2, :],
                                perf_mode=mybir.MatmulPerfMode.DoubleRow,
                                start=False, stop=False,
                            )
                    else:
                        for fc in range(FC):
                            nc.tensor.matmul(
                                ps[:], w1bt_t[:, fc, nsl], bt[:, fc, :],
                                start=(fc == 0), stop=False,
                            )
                        for fc in range(FC):
                            nc.tensor.matmul(
                                ps[:], w1abt_t[:, fc, nsl], abt[:, fc, :],
                                start=False, stop=False,
                            )
                    nc.tensor.matmul(
                        ps[:], wg[:, nsl],
                        pwt_t[:, g * GRP:(g + 1) * GRP],
                        start=False, stop=True,
                    )
                    ht = htp.tile([128, GRP], BF16)
                    nc.scalar.activation(
                        ht[:], ps[:],
                        mybir.ActivationFunctionType.Lrelu, alpha=0.01,
                        scale=(1.0 / FP8_SCALE) if USE_FP8 else 1.0,
                    )
                    hts.append(ht)
                    if nt == 1 and prev_group is not None:
                        p_g, p_ps4, p_hts = prev_group
                        emit_batch(p_ps4, p_hts[4:8], range(4, 8), start=False)
                        finalize_group(p_ps4, p_g, rtiles.pop(p_g))
                        prev_group = None
                    if nt == 5:
                        ps4 = psum_s_pool.tile([128, GRP], F32)
                        emit_batch(ps4, hts[0:4], range(0, 4), start=True)
                prev_group = (g, ps4, hts)
            # flush the last group's second batch
            p_g, p_ps4, p_hts = prev_group
            emit_batch(p_ps4, p_hts[4:8], range(4, 8), start=False)
            finalize_group(p_ps4, p_g, rtiles.pop(p_g))

    nc.compile()
    return nc


def prep_inputs(all_mentions, mentions_batch, pw_batch, top_indices_batch,
                top_rough_scores_batch, W1, b1, Wout, bout, n_cores=N_CORES):
    """Host-side marshalling: shard over batch, cast/transpose into the
    layouts the kernel expects. Returns (in_maps, B, n_tab, bout_val)."""
    bf = ml_dtypes.bfloat16
    batch = mentions_batch.shape[0]
    B = batch // n_cores
    n_tab = all_mentions.shape[0]
    FC = EMB // 128
    NT = HID // 128
    G = (B * N_ANTS) // GRP

    amen = np.ascontiguousarray(all_mentions.astype(bf))

    def wt_block(Wcols, scale=1.0, dtype=bf):
        # [1024, 1024] f32 block -> [128, FC, HID] (feature on partitions)
        wt = Wcols.T.reshape(FC, 128, HID).transpose(1, 0, 2) * scale
        if dtype is not bf:
            wt = np.clip(wt, -240.0, 240.0)
        return np.ascontiguousarray(wt.astype(dtype))

    S = FP8_SCALE if USE_FP8 else 1.0
    f8 = ml_dtypes.float8_e4m3
    wdt = f8 if USE_FP8 else bf
    w1at = wt_block(W1[:, 0:EMB], S)
    w1bt = wt_block(W1[:, EMB:2 * EMB], S, wdt)
    w1abt = wt_block(W1[:, 2 * EMB:3 * EMB], S, wdt)
    w1pw = np.zeros((128, HID), dtype=bf)
    w1pw[:PW] = (W1[:, 3 * EMB:3 * EMB + PW].T * S).astype(bf)
    w1pw[72] = (b1 * S).astype(bf)
    woutt = np.ascontiguousarray(Wout[0].reshape(NT, 128).T.astype(bf))

    in_maps = []
    for c in range(n_cores):
        rows = slice(c * B, (c + 1) * B)
        m_c = np.asarray(mentions_batch[rows], dtype=np.float32)       # [B, 1024]
        ment = np.ascontiguousarray(
            m_c.T.reshape(FC, 128, B).transpose(1, 0, 2).astype(bf))   # [128, FC, B]
        pw_c = np.asarray(pw_batch[rows], dtype=np.float32)            # [B, 64, 64]
        pwt = np.zeros((128, B * N_ANTS), dtype=bf)
        pwt[:PW] = pw_c.reshape(B * N_ANTS, PW).T.astype(bf)
        cols = np.arange(B * N_ANTS)
        for q in range(ROWS_PER_GRP):
            pwt[PW + q] = ((cols % GRP) // N_ANTS == q).astype(bf)
        pwt[72] = np.ones(B * N_ANTS, dtype=bf)
        idx_c = np.asarray(top_indices_batch[rows]).astype(np.int64).reshape(-1)
        idx_tiles = []
        for g in range(G):
            v = idx_c[g * GRP:(g + 1) * GRP].astype(np.int16)
            idx_tiles.append(np.tile(v.reshape(GRP // 16, 16).T, (8, 1)))
        idx = np.ascontiguousarray(np.concatenate(idx_tiles, axis=1))  # [128, G*32]
        rough = np.ascontiguousarray(
            np.asarray(top_rough_scores_batch[rows], dtype=np.float32).reshape(1, -1)
            + np.float32(np.asarray(bout).reshape(-1)[0]))
        in_maps.append({
            "amen": amen, "ment": ment, "w1bt": w1bt, "w1abt": w1abt,
            "w1at": w1at, "w1pw": w1pw, "woutt": woutt,
            "pwt": pwt, "idx": idx, "rough": rough,
        })
    return in_maps, B, n_tab


_NC_CACHE = {}


def kernel_with_results(all_mentions, mentions_batch, pw_batch, top_indices_batch,
                        top_rough_scores_batch, W1, b1, Wout, bout, **run_kwargs):
    args = [np.asarray(x) for x in (
        all_mentions, mentions_batch, pw_batch, top_indices_batch,
        top_rough_scores_batch, W1, b1, Wout, bout)]
    in_maps, B, n_tab = prep_inputs(*args)
    assert n_tab < 32768, "gather indices are int16"
    key = (B, n_tab)
    if key not in _NC_CACHE:
        _NC_CACHE[key] = build_nc(B, n_tab)
    nc = _NC_CACHE[key]
    res = None
    for attempt in range(3):
        try:
            res = run_bass_kernel_spmd(nc, in_maps, list(range(N_CORES)), **run_kwargs)
            break
        except Exception:
            if attempt == 2:
                raise
            import time
            time.sleep(5)
    scores = np.concatenate([np.asarray(r["out"]) for r in res.results], axis=0)
    batch = scores.shape[0]
    full = np.empty((batch, N_ANTS + 1), np.float32)
    full[:, 0] = EPS
    full[:, 1:] = scores
    return full, res


def kernel(**inputs) -> np.ndarray:
    out, _ = kernel_with_results(**inputs)
    return out


# revision 17
# speedup vs baseline: 1.0619x; 1.0034x over previous
"""Trainium2 Bass kernel for the AnaphoricityScorer (coref pairwise FFNN scorer).

Math (per batch row i, antecedent slot t):
    b  = all_mentions[top_indices[i, t]]                    # gathered mention
    pair = [a_i, b, a_i * b, pw[i, t]]                      # 3*1024 + 64 features
    h  = leaky_relu(pair @ W1.T + b1, 0.01)                 # 1024 hidden
    ffnn = h @ Wout.T + bout                                # scalar
    score = rough[i, t] + ffnn
    out = concat([eps_col, scores], axis=1)                 # [batch, 65]

Distribution: pure data parallel over the batch dim across 8 NeuronCores
(no collectives). all_mentions and FFNN weights are replicated.

Per-core algorithm (B = 128 batch rows -> 8192 pair rows, groups of 512):
  - The a-term a_i @ W1a.T is identical for all 64 antecedents of row i, so it
    is computed once per batch row in a prologue (ha = mentions @ W1a.T + b1)
    and broadcast-added into the pair-row PSUM with a stride-0 DVE add.
  - b rows arrive transposed (features on partitions) straight from HBM via
    dma_gather(transpose=True), which is exactly the matmul rhs layout.
  - a*b is built by a DVE multiply against a stride-0 broadcast of mentions^T.
  - One PSUM accumulation of 17 matmuls per (row-group, hidden-tile):
    8 K-tiles of W1b, 8 of W1ab, 1 of W1pw (K=64).
  - Lrelu on ScalarE evicts PSUM -> SBUF bf16; the Wout reduction is a
    K=128, M=1 matmul accumulated over the 8 hidden tiles.
  - bout + rough scores are added on DVE; one DMA out per core.

Everything is bf16 on the TensorEngine with fp32 PSUM accumulation.
"""

import sys

for _p in ("/opt/trn_rl_repo",):
    if _p not in sys.path:
        sys.path.append(_p)

import numpy as np
import ml_dtypes

import concourse.bacc as bacc
import concourse.mybir as mybir
from concourse.tile import TileContext
from concourse.bass_utils import run_bass_kernel_spmd

BF16 = mybir.dt.bfloat16
F32 = mybir.dt.float32
I16 = mybir.dt.int16
FP8 = mybir.dt.float8e4

USE_FP8 = True       # b/ab blocks in fp8-e4m3 DoubleRow (2 k-tiles per matmul)
FP8_SCALE = 512.0    # weight pre-scale so 0.02-magnitude weights leave fp8 denormals

N_CORES = 8
EMB = 1024
HID = 1024
N_ANTS = 64
PW = 64
EPS = 1e-7
GRP = 512          # pair rows per group (= 8 batch rows)
ROWS_PER_GRP = 8   # batch rows per group


def build_nc(B: int, n_tab: int):
    """Build the per-core Bass graph. B = batch rows per core."""
    G = (B * N_ANTS) // GRP  # number of row groups
    FC = EMB // 128          # 8 feature k-tiles per 1024-feature block
    NT = HID // 128          # 8 hidden tiles

    nc = bacc.Bacc("TRN2")
    amen = nc.declare_dram_parameter("amen", [n_tab, EMB], BF16, isOutput=False)
    ment = nc.declare_dram_parameter("ment", [128, FC, B], BF16, isOutput=False)
    wdt = FP8 if USE_FP8 else BF16
    w1bt = nc.declare_dram_parameter("w1bt", [128, FC, HID], wdt, isOutput=False)
    w1abt = nc.declare_dram_parameter("w1abt", [128, FC, HID], wdt, isOutput=False)
    w1at = nc.declare_dram_parameter("w1at", [128, FC, HID], BF16, isOutput=False)
    w1pw = nc.declare_dram_parameter("w1pw", [128, HID], BF16, isOutput=False)
    woutt = nc.declare_dram_parameter("woutt", [128, NT], BF16, isOutput=False)
    pwt = nc.declare_dram_parameter("pwt", [128, B * N_ANTS], BF16, isOutput=False)
    idx = nc.declare_dram_parameter("idx", [128, G * (GRP // 16)], I16, isOutput=False)
    rough = nc.declare_dram_parameter("rough", [1, B * N_ANTS], F32, isOutput=False)
    out = nc.declare_dram_parameter("out", [B, N_ANTS], F32, isOutput=True)

    with TileContext(nc) as tc:
        with (
            tc.tile_pool(name="const", bufs=1) as const,
            tc.tile_pool(name="btp", bufs=5) as btp,
            tc.tile_pool(name="abtp", bufs=4) as abtp,
            tc.tile_pool(name="bt8p", bufs=4) as bt8p,
            tc.tile_pool(name="wgp", bufs=3) as wgp,
            tc.tile_pool(name="htp", bufs=10) as htp,
            tc.tile_pool(name="rpool", bufs=3) as rpool,
            tc.tile_pool(name="spool", bufs=2) as spool,
            tc.tile_pool(name="psum", bufs=4, space="PSUM") as psum_pool,
            tc.tile_pool(name="psum_s", bufs=2, space="PSUM") as psum_s_pool,
        ):
            # ---- resident loads (gather + prologue deps first) ------------
            idx_t = const.tile([128, G * (GRP // 16)], I16)
            nc.sync.dma_start(idx_t[:], idx[:, :])
            ment_t = const.tile([128, FC, B], BF16)
            nc.sync.dma_start(ment_t[:], ment[:, :, :])
            w1at_t = const.tile([128, FC, HID], BF16)
            nc.sync.dma_start(w1at_t[:], w1at[:, :, :])
            w1bt_t = const.tile([128, FC, HID], wdt)
            nc.sync.dma_start(w1bt_t[:], w1bt[:, :, :])
            w1abt_t = const.tile([128, FC, HID], wdt)
            nc.sync.dma_start(w1abt_t[:], w1abt[:, :, :])
            w1pw_t = const.tile([128, HID], BF16)
            nc.sync.dma_start(w1pw_t[:], w1pw[:, :])
            woutt_t = const.tile([128, NT], BF16)
            nc.sync.dma_start(woutt_t[:], woutt[:, :])
            pwt_t = const.tile([128, B * N_ANTS], BF16)
            nc.sync.dma_start(pwt_t[:], pwt[:, :])
            # ---- prologue: ha = mentions @ (W1a*S).T, rows-on-partitions --
            # ha2r regroups ha so group g's 8 batch rows sit on partitions
            # 64..71 of the per-group weight tile wg (spliced below); the
            # static pwt operand carries one-hot rows that select the batch
            # row, folding the a-term (and b1 via an all-ones row) into the
            # pw matmul for free.
            ha2 = const.tile([B, HID], BF16)
            for half in range(HID // 512):
                pp = psum_s_pool.tile([B, 512], F32)
                for fc in range(FC):
                    nc.tensor.matmul(
                        pp[:],
                        ment_t[:, fc, :],
                        w1at_t[:, fc, half * 512:(half + 1) * 512],
                        start=(fc == 0),
                        stop=(fc == FC - 1),
                    )
                nc.scalar.activation(
                    ha2[:, half * 512:(half + 1) * 512], pp[:],
                    mybir.ActivationFunctionType.Identity,
                )
            ha2_dram = nc.dram_tensor("ha2_scratch", [B, HID], BF16)
            nc.sync.dma_start(ha2_dram[:, :], ha2[:])
            ha2r = const.tile([8, G, HID], BF16)
            nc.sync.dma_start(
                ha2r[:],
                ha2_dram[:, :].rearrange("(g q) n -> q g n", q=ROWS_PER_GRP),
            )

            # ---- main loop over row groups --------------------------------
            # Software-pipelined emission: the gather + a*b multiplies for
            # group g+1 are emitted BEFORE group g's matmuls so the DVE
            # stream reaches them early, and each (g, nt) second-matmul is
            # deferred by one nt so its ht dependency never stalls PE.
            def produce_group(g):
                r0 = g * ROWS_PER_GRP
                rtile = rpool.tile([1, GRP], F32)
                nc.sync.dma_start(rtile[:], rough[0:1, g * GRP:(g + 1) * GRP])
                bt = btp.tile([128, FC, GRP], BF16)
                nc.gpsimd.dma_gather(
                    bt[:], amen[:, :],
                    idx_t[:, g * (GRP // 16):(g + 1) * (GRP // 16)],
                    GRP, GRP, EMB, transpose=True,
                )
                abt = abtp.tile([128, FC, GRP], FP8 if USE_FP8 else BF16)
                a_b = ment_t[:, :, r0:r0 + ROWS_PER_GRP]
                for fc in range(FC):
                    nc.vector.tensor_mul(
                        abt[:, fc, :].rearrange("p (a b) -> p a b", a=ROWS_PER_GRP),
                        bt[:, fc, :].rearrange("p (a b) -> p a b", a=ROWS_PER_GRP),
                        a_b[:, fc, :].unsqueeze(2).to_broadcast(
                            [128, ROWS_PER_GRP, N_ANTS]),
                    )
                if USE_FP8:
                    bt8 = bt8p.tile([128, FC, GRP], FP8)
                    for fc in range(FC):
                        nc.scalar.activation(
                            bt8[:, fc, :], bt[:, fc, :],
                            mybir.ActivationFunctionType.Identity)
                    bt = bt8
                wg = wgp.tile([128, HID], BF16)
                nc.vector.tensor_copy(wg[:], w1pw_t[:])
                nc.vector.tensor_copy(wg[64:72, :], ha2r[:, g, :])
                return bt, abt, rtile, wg

            def emit_batch(ps4, hts, nts, start):
                # 4 M=1 matmuls packed into distinct PE column groups -- they
                # execute concurrently in the array (one per 32-col strip)
                for nt_i, ht_i in zip(nts, hts):
                    j = nt_i % 4
                    nc.tensor.matmul(
                        ps4[32 * j:32 * j + 1, :], woutt_t[:, nt_i:nt_i + 1],
                        ht_i[:], tile_position=(0, 32 * j),
                        start=start, stop=not start,
                    )

            def finalize_group(ps4, p_g, p_rtile):
                # DVE may read at most one PSUM operand per op: chain the four
                # column-group partial rows through SBUF
                t1 = spool.tile([1, GRP], F32)
                nc.vector.tensor_add(t1[:], ps4[0:1, :], p_rtile[:])
                t2 = spool.tile([1, GRP], F32)
                nc.vector.tensor_add(t2[:], ps4[32:33, :], t1[:])
                t3 = spool.tile([1, GRP], F32)
                nc.vector.tensor_add(t3[:], ps4[64:65, :], t2[:])
                stile = spool.tile([1, GRP], F32)
                nc.vector.tensor_add(stile[:], ps4[96:97, :], t3[:])
                nc.sync.dma_start(
                    out[p_g * ROWS_PER_GRP:(p_g + 1) * ROWS_PER_GRP, :].unsqueeze(0),
                    stile[:].rearrange("p (r c) -> p r c", r=ROWS_PER_GRP),
                )

            tiles = {0: produce_group(0)}
            rtiles = {}
            prev_group = None  # (g, ps4, hts) awaiting its second batch
            for g in range(G):
                r0 = g * ROWS_PER_GRP
                bt, abt, rtiles[g], wg = tiles.pop(g)
                if g + 1 < G:
                    tiles[g + 1] = produce_group(g + 1)
                hts = []
                ps4 = None
                for nt in range(NT):
                    ps = psum_pool.tile([128, GRP], F32)
                    nsl = slice(nt * 128, (nt + 1) * 128)
                    if USE_FP8:
                        for fc in range(0, FC, 2):
                            nc.tensor.matmul(
                                ps[:], w1bt_t[:, fc:fc + 2, nsl], bt[:, fc:fc + 2, :],
                                perf_mode=mybir.MatmulPerfMode.DoubleRow,
                                start=(fc == 0), stop=False,
                            )
                        for fc in range(0, FC, 2):
                            nc.tensor.matmul(
                                ps[:], w1abt_t[:, fc:fc + 2, nsl], abt[:, fc:fc + 2, :],
                                perf_mode=mybir.MatmulPerfMode.DoubleRow,
                                start=False, stop=False,
                            )
                    else:
                        for fc in range(FC):
                            nc.tensor.matmul(
                                ps[:], w1bt_t[:, fc, nsl], bt[:, fc, :],
                                start=(fc == 0), stop=False,
                            )
                        for fc in range(FC):
                            nc.tensor.matmul(
                                ps[:], w1abt_t[:, fc, nsl], abt[:, fc, :],
                                start=False, stop=False,
                            )
                    nc.tensor.matmul(
                        ps[:], wg[:, nsl],
                        pwt_t[:, g * GRP:(g + 1) * GRP],
                        start=False, stop=True,
                    )
                    ht = htp.tile([128, GRP], BF16)
                    nc.scalar.activation(
                        ht[:], ps[:],
                        mybir.ActivationFunctionType.Lrelu, alpha=0.01,
                        scale=(1.0 / FP8_SCALE) if USE_FP8 else 1.0,
                    )
                    hts.append(ht)
                    if nt == 1 and prev_group is not None:
                        p_g, p_ps4, p_hts = prev_group
                        emit_batch(p_ps4, p_hts[4:8], range(4, 8), start=False)
                        finalize_group(p_ps4, p_g, rtiles.pop(p_g))
                        prev_group = None
                    if nt == 5:
                        ps4 = psum_s_pool.tile([128, GRP], F32)
                        emit_batch(ps4, hts[0:4], range(0, 4), start=True)
                prev_group = (g, ps4, hts)
            # flush the last group's second batch
            p_g, p_ps4, p_hts = prev_group
            emit_batch(p_ps4, p_hts[4:8], range(4, 8), start=False)
            finalize_group(p_ps4, p_g, rtiles.pop(p_g))

    nc.compile()
    return nc


def prep_inputs(all_mentions, mentions_batch, pw_batch, top_indices_batch,
                top_rough_scores_batch, W1, b1, Wout, bout, n_cores=N_CORES):
    """Host-side marshalling: shard over batch, cast/transpose into the
    layouts the kernel expects. Returns (in_maps, B, n_tab, bout_val)."""
    bf = ml_dtypes.bfloat16
    batch = mentions_batch.shape[0]
    B = batch // n_cores
    n_tab = all_mentions.shape[0]
    FC = EMB // 128
    NT = HID // 128
    G = (B * N_ANTS) // GRP

    amen = np.ascontiguousarray(all_mentions.astype(bf))

    def wt_block(Wcols, scale=1.0, dtype=bf):
        # [1024, 1024] f32 block -> [128, FC, HID] (feature on partitions)
        wt = Wcols.T.reshape(FC, 128, HID).transpose(1, 0, 2) * scale
        if dtype is not bf:
            wt = np.clip(wt, -240.0, 240.0)
        return np.ascontiguousarray(wt.astype(dtype))

    S = FP8_SCALE if USE_FP8 else 1.0
    f8 = ml_dtypes.float8_e4m3
    wdt = f8 if USE_FP8 else bf
    w1at = wt_block(W1[:, 0:EMB], S)
    w1bt = wt_block(W1[:, EMB:2 * EMB], S, wdt)
    w1abt = wt_block(W1[:, 2 * EMB:3 * EMB], S, wdt)
    w1pw = np.zeros((128, HID), dtype=bf)
    w1pw[:PW] = (W1[:, 3 * EMB:3 * EMB + PW].T * S).astype(bf)
    w1pw[72] = (b1 * S).astype(bf)
    woutt = np.ascontiguousarray(Wout[0].reshape(NT, 128).T.astype(bf))

    in_maps = []
    for c in range(n_cores):
        rows = slice(c * B, (c + 1) * B)
        m_c = np.asarray(mentions_batch[rows], dtype=np.float32)       # [B, 1024]
        ment = np.ascontiguousarray(
            m_c.T.reshape(FC, 128, B).transpose(1, 0, 2).astype(bf))   # [128, FC, B]
        pw_c = np.asarray(pw_batch[rows], dtype=np.float32)            # [B, 64, 64]
        pwt = np.zeros((128, B * N_ANTS), dtype=bf)
        pwt[:PW] = pw_c.reshape(B * N_ANTS, PW).T.astype(bf)
        cols = np.arange(B * N_ANTS)
        for q in range(ROWS_PER_GRP):
            pwt[PW + q] = ((cols % GRP) // N_ANTS == q).astype(bf)
        pwt[72] = np.ones(B * N_ANTS, dtype=bf)
        idx_c = np.asarray(top_indices_batch[rows]).astype(np.int64).reshape(-1)
        idx_tiles = []
        for g in range(G):
            v = idx_c[g * GRP:(g + 1) * GRP].astype(np.int16)
            idx_tiles.append(np.tile(v.reshape(GRP // 16, 16).T, (8, 1)))
        idx = np.ascontiguousarray(np.concatenate(idx_tiles, axis=1))  # [128, G*32]
        rough = np.ascontiguousarray(
            np.asarray(top_rough_scores_batch[rows], dtype=np.float32).reshape(1, -1)
            + np.float32(np.asarray(bout).reshape(-1)[0]))
        in_maps.append({
            "amen": amen, "ment": ment, "w1bt": w1bt, "w1abt": w1abt,
            "w1at": w1at, "w1pw": w1pw, "woutt": woutt,
            "pwt": pwt, "idx": idx, "rough": rough,
        })
    return in_maps, B, n_tab


_NC_CACHE = {}


def kernel_with_results(all_mentions, mentions_batch, pw_batch, top_indices_batch,
                        top_rough_scores_batch, W1, b1, Wout, bout, **run_kwargs):
    args = [np.asarray(x) for x in (
        all_mentions, mentions_batch, pw_batch, top_indices_batch,
        top_rough_scores_batch, W1, b1, Wout, bout)]
    in_maps, B, n_tab = prep_inputs(*args)
    assert n_tab < 32768, "gather indices are int16"
    key = (B, n_tab)
    if key not in _NC_CACHE:
        _NC_CACHE[key] = build_nc(B, n_tab)
    nc = _NC_CACHE[key]
    res = None
    for attempt in range(3):
        try:
            res = run_bass_kernel_spmd(nc, in_maps, list(range(N_CORES)), **run_kwargs)
            break
        except Exception:
            if attempt == 2:
                raise
            import time
            time.sleep(5)
    scores = np.concatenate([np.asarray(r["out"]) for r in res.results], axis=0)
    batch = scores.shape[0]
    full = np.empty((batch, N_ANTS + 1), np.float32)
    full[:, 0] = EPS
    full[:, 1:] = scores
    return full, res


def kernel(**inputs) -> np.ndarray:
    out, _ = kernel_with_results(**inputs)
    return out


# revision 18
# speedup vs baseline: 1.2394x; 1.1671x over previous
"""Trainium2 Bass kernel for the AnaphoricityScorer (coref pairwise FFNN scorer).

Math (per batch row i, antecedent slot t):
    b  = all_mentions[top_indices[i, t]]                    # gathered mention
    pair = [a_i, b, a_i * b, pw[i, t]]                      # 3*1024 + 64 features
    h  = leaky_relu(pair @ W1.T + b1, 0.01)                 # 1024 hidden
    ffnn = h @ Wout.T + bout                                # scalar
    score = rough[i, t] + ffnn
    out = concat([eps_col, scores], axis=1)                 # [batch, 65]

Distribution: pure data parallel over the batch dim across 8 NeuronCores
(no collectives). all_mentions and FFNN weights are replicated.

Per-core algorithm (B = 128 batch rows -> 8192 pair rows, groups of 512):
  - b rows arrive transposed (features on partitions) straight from HBM via
    dma_gather(transpose=True), which is exactly the matmul rhs layout.
  - a*b is built by a DVE multiply against a stride-0 broadcast of mentions^T,
    written directly as fp8; b is cast bf16->fp8 on ScalarE.
  - The W1b / W1ab blocks run as fp8-e4m3 DoubleRow matmuls (two 128-feature
    k-tiles per instruction, 2 MACs/cell/cycle): 4 + 4 matmuls per
    (row-group, hidden-tile) instead of 16 bf16 ones. Weights are pre-scaled
    by FP8_SCALE on the host so 0.02-magnitude values clear fp8 denormals;
    the descale rides the Lrelu eviction's `scale` for free.
  - The a-term (a_i @ W1a.T, shared by all 64 antecedents of batch row i) and
    b1 are folded into the 9th (pw) matmul: its K=128 stationary tile carries
    W1pw in rows 0..63, the 8 per-group ha rows in 64..71 and b1 in row 72,
    while the static moving operand has matching one-hot / all-ones rows.
  - Lrelu on ScalarE evicts PSUM -> SBUF bf16 (applying 1/FP8_SCALE); the
    Wout reduction runs as col-tiled M=1 matmuls, 4 packed per PE pass via
    tile_position, deferred so they never stall the main pipeline.
  - Emission is software-pipelined one group ahead (gather + multiplies for
    group g+1 precede group g's matmuls) with deep tile pools so the PE
    stream never waits on SWDGE gathers.
"""

import sys

for _p in ("/opt/trn_rl_repo",):
    if _p not in sys.path:
        sys.path.append(_p)

import numpy as np
import ml_dtypes

import concourse.bacc as bacc
import concourse.mybir as mybir
from concourse.tile import TileContext
from concourse.bass_utils import run_bass_kernel_spmd

BF16 = mybir.dt.bfloat16
F32 = mybir.dt.float32
I16 = mybir.dt.int16
FP8 = mybir.dt.float8e4

USE_FP8 = True       # b/ab blocks in fp8-e4m3 DoubleRow (2 k-tiles per matmul)
FP8_SCALE = 512.0    # weight pre-scale so 0.02-magnitude weights leave fp8 denormals

N_CORES = 8
EMB = 1024
HID = 1024
N_ANTS = 64
PW = 64
EPS = 1e-7
GRP = 512          # pair rows per group (= 8 batch rows)
ROWS_PER_GRP = 8   # batch rows per group


def build_nc(B: int, n_tab: int):
    """Build the per-core Bass graph. B = batch rows per core."""
    G = (B * N_ANTS) // GRP  # number of row groups
    FC = EMB // 128          # 8 feature k-tiles per 1024-feature block
    NT = HID // 128          # 8 hidden tiles

    nc = bacc.Bacc("TRN2")
    amen = nc.declare_dram_parameter("amen", [n_tab, EMB], BF16, isOutput=False)
    ment = nc.declare_dram_parameter("ment", [128, FC, B], BF16, isOutput=False)
    wdt = FP8 if USE_FP8 else BF16
    w1bt = nc.declare_dram_parameter("w1bt", [128, FC, HID], wdt, isOutput=False)
    w1abt = nc.declare_dram_parameter("w1abt", [128, FC, HID], wdt, isOutput=False)
    w1at = nc.declare_dram_parameter("w1at", [128, FC, HID], BF16, isOutput=False)
    w1pw = nc.declare_dram_parameter("w1pw", [128, HID], BF16, isOutput=False)
    woutt = nc.declare_dram_parameter("woutt", [128, NT], BF16, isOutput=False)
    pwt = nc.declare_dram_parameter("pwt", [128, B * N_ANTS], BF16, isOutput=False)
    idx = nc.declare_dram_parameter("idx", [128, G * (GRP // 16)], I16, isOutput=False)
    rough = nc.declare_dram_parameter("rough", [1, B * N_ANTS], F32, isOutput=False)
    out = nc.declare_dram_parameter("out", [B, N_ANTS], F32, isOutput=True)

    with TileContext(nc) as tc:
        with (
            tc.tile_pool(name="const", bufs=1) as const,
            tc.tile_pool(name="btp", bufs=5) as btp,
            tc.tile_pool(name="abtp", bufs=4) as abtp,
            tc.tile_pool(name="bt8p", bufs=4) as bt8p,
            tc.tile_pool(name="wgp", bufs=3) as wgp,
            tc.tile_pool(name="htp", bufs=10) as htp,
            tc.tile_pool(name="rpool", bufs=3) as rpool,
            tc.tile_pool(name="spool", bufs=2) as spool,
            tc.tile_pool(name="psum", bufs=4, space="PSUM") as psum_pool,
            tc.tile_pool(name="psum_s", bufs=2, space="PSUM") as psum_s_pool,
        ):
            # ---- resident loads (gather + prologue deps first) ------------
            idx_t = const.tile([128, G * (GRP // 16)], I16)
            nc.sync.dma_start(idx_t[:], idx[:, :])
            ment_t = const.tile([128, FC, B], BF16)
            nc.sync.dma_start(ment_t[:], ment[:, :, :])
            w1at_t = const.tile([128, FC, HID], BF16)
            nc.sync.dma_start(w1at_t[:], w1at[:, :, :])
            w1bt_t = const.tile([128, FC, HID], wdt)
            nc.sync.dma_start(w1bt_t[:], w1bt[:, :, :])
            w1abt_t = const.tile([128, FC, HID], wdt)
            nc.sync.dma_start(w1abt_t[:], w1abt[:, :, :])
            w1pw_t = const.tile([128, HID], BF16)
            nc.sync.dma_start(w1pw_t[:], w1pw[:, :])
            woutt_t = const.tile([128, NT], BF16)
            nc.sync.dma_start(woutt_t[:], woutt[:, :])
            pwt_t = const.tile([128, B * N_ANTS], BF16)
            nc.sync.dma_start(pwt_t[:], pwt[:, :])
            # ---- prologue: ha = mentions @ (W1a*S).T, rows-on-partitions --
            # ha2r regroups ha so group g's 8 batch rows sit on partitions
            # 64..71 of the per-group weight tile wg (spliced below); the
            # static pwt operand carries one-hot rows that select the batch
            # row, folding the a-term (and b1 via an all-ones row) into the
            # pw matmul for free.
            ha2 = const.tile([B, HID], BF16)
            for half in range(HID // 512):
                pp = psum_s_pool.tile([B, 512], F32)
                for fc in range(FC):
                    nc.tensor.matmul(
                        pp[:],
                        ment_t[:, fc, :],
                        w1at_t[:, fc, half * 512:(half + 1) * 512],
                        start=(fc == 0),
                        stop=(fc == FC - 1),
                    )
                nc.scalar.activation(
                    ha2[:, half * 512:(half + 1) * 512], pp[:],
                    mybir.ActivationFunctionType.Identity,
                )
            ha2_dram = nc.dram_tensor("ha2_scratch", [B, HID], BF16)
            nc.sync.dma_start(ha2_dram[:, :], ha2[:])
            ha2r = const.tile([8, G, HID], BF16)
            nc.sync.dma_start(
                ha2r[:],
                ha2_dram[:, :].rearrange("(g q) n -> q g n", q=ROWS_PER_GRP),
            )

            # ---- main loop over row groups --------------------------------
            # Software-pipelined emission: the gather + a*b multiplies for
            # group g+1 are emitted BEFORE group g's matmuls so the DVE
            # stream reaches them early, and each (g, nt) second-matmul is
            # deferred by one nt so its ht dependency never stalls PE.
            def produce_group(g):
                r0 = g * ROWS_PER_GRP
                rtile = rpool.tile([1, GRP], F32)
                nc.sync.dma_start(rtile[:], rough[0:1, g * GRP:(g + 1) * GRP])
                bt = btp.tile([128, FC, GRP], BF16)
                nc.gpsimd.dma_gather(
                    bt[:], amen[:, :],
                    idx_t[:, g * (GRP // 16):(g + 1) * (GRP // 16)],
                    GRP, GRP, EMB, transpose=True,
                )
                abt = abtp.tile([128, FC, GRP], FP8 if USE_FP8 else BF16)
                a_b = ment_t[:, :, r0:r0 + ROWS_PER_GRP]
                for fc in range(FC):
                    nc.vector.tensor_mul(
                        abt[:, fc, :].rearrange("p (a b) -> p a b", a=ROWS_PER_GRP),
                        bt[:, fc, :].rearrange("p (a b) -> p a b", a=ROWS_PER_GRP),
                        a_b[:, fc, :].unsqueeze(2).to_broadcast(
                            [128, ROWS_PER_GRP, N_ANTS]),
                    )
                if USE_FP8:
                    bt8 = bt8p.tile([128, FC, GRP], FP8)
                    for fc in range(FC):
                        nc.scalar.activation(
                            bt8[:, fc, :], bt[:, fc, :],
                            mybir.ActivationFunctionType.Identity)
                    bt = bt8
                wg = wgp.tile([128, HID], BF16)
                nc.vector.tensor_copy(wg[:], w1pw_t[:])
                nc.vector.tensor_copy(wg[64:72, :], ha2r[:, g, :])
                return bt, abt, rtile, wg

            def emit_batch(ps4, hts, nts, start):
                # 4 M=1 matmuls packed into distinct PE column groups -- they
                # execute concurrently in the array (one per 32-col strip)
                for nt_i, ht_i in zip(nts, hts):
                    j = nt_i % 4
                    nc.tensor.matmul(
                        ps4[32 * j:32 * j + 1, :], woutt_t[:, nt_i:nt_i + 1],
                        ht_i[:], tile_position=(0, 32 * j),
                        start=start, stop=not start,
                    )

            def finalize_group(ps4, p_g, p_rtile):
                # DVE may read at most one PSUM operand per op: chain the four
                # column-group partial rows through SBUF
                t1 = spool.tile([1, GRP], F32)
                nc.vector.tensor_add(t1[:], ps4[0:1, :], p_rtile[:])
                t2 = spool.tile([1, GRP], F32)
                nc.vector.tensor_add(t2[:], ps4[32:33, :], t1[:])
                t3 = spool.tile([1, GRP], F32)
                nc.vector.tensor_add(t3[:], ps4[64:65, :], t2[:])
                stile = spool.tile([1, GRP], F32)
                nc.vector.tensor_add(stile[:], ps4[96:97, :], t3[:])
                nc.sync.dma_start(
                    out[p_g * ROWS_PER_GRP:(p_g + 1) * ROWS_PER_GRP, :].unsqueeze(0),
                    stile[:].rearrange("p (r c) -> p r c", r=ROWS_PER_GRP),
                )

            tiles = {0: produce_group(0)}
            rtiles = {}
            prev_group = None  # (g, ps4, hts) awaiting its second batch
            for g in range(G):
                r0 = g * ROWS_PER_GRP
                bt, abt, rtiles[g], wg = tiles.pop(g)
                if g + 1 < G:
                    tiles[g + 1] = produce_group(g + 1)
                hts = []
                ps4 = None
                for nt in range(NT):
                    ps = psum_pool.tile([128, GRP], F32)
                    nsl = slice(nt * 128, (nt + 1) * 128)
                    if USE_FP8:
                        for fc in range(0, FC, 2):
                            nc.tensor.matmul(
                                ps[:], w1bt_t[:, fc:fc + 2, nsl], bt[:, fc:fc + 2, :],
                                perf_mode=mybir.MatmulPerfMode.DoubleRow,
                                start=(fc == 0), stop=False,
                            )
                        for fc in range(0, FC, 2):
                            nc.tensor.matmul(
                                ps[:], w1abt_t[:, fc:fc + 2, nsl], abt[:, fc:fc + 2, :],
                                perf_mode=mybir.MatmulPerfMode.DoubleRow,
                                start=False, stop=False,
                            )
                    else:
                        for fc in range(FC):
                            nc.tensor.matmul(
                                ps[:], w1bt_t[:, fc, nsl], bt[:, fc, :],
                                start=(fc == 0), stop=False,
                            )
                        for fc in range(FC):
                            nc.tensor.matmul(
                                ps[:], w1abt_t[:, fc, nsl], abt[:, fc, :],
                                start=False, stop=False,
                            )
                    nc.tensor.matmul(
                        ps[:], wg[:, nsl],
                        pwt_t[:, g * GRP:(g + 1) * GRP],
                        start=False, stop=True,
                    )
                    ht = htp.tile([128, GRP], BF16)
                    nc.scalar.activation(
                        ht[:], ps[:],
                        mybir.ActivationFunctionType.Lrelu, alpha=0.01,
                        scale=(1.0 / FP8_SCALE) if USE_FP8 else 1.0,
                    )
                    hts.append(ht)
                    if nt == 1 and prev_group is not None:
                        p_g, p_ps4, p_hts = prev_group
                        emit_batch(p_ps4, p_hts[4:8], range(4, 8), start=False)
                        finalize_group(p_ps4, p_g, rtiles.pop(p_g))
                        prev_group = None
                    if nt == 5:
                        ps4 = psum_s_pool.tile([128, GRP], F32)
                        emit_batch(ps4, hts[0:4], range(0, 4), start=True)
                prev_group = (g, ps4, hts)
            # flush the last group's second batch
            p_g, p_ps4, p_hts = prev_group
            emit_batch(p_ps4, p_hts[4:8], range(4, 8), start=False)
            finalize_group(p_ps4, p_g, rtiles.pop(p_g))

    nc.compile()
    return nc


def prep_inputs(all_mentions, mentions_batch, pw_batch, top_indices_batch,
                top_rough_scores_batch, W1, b1, Wout, bout, n_cores=N_CORES):
    """Host-side marshalling: shard over batch, cast/transpose into the
    layouts the kernel expects. Returns (in_maps, B, n_tab, bout_val)."""
    bf = ml_dtypes.bfloat16
    batch = mentions_batch.shape[0]
    B = batch // n_cores
    n_tab = all_mentions.shape[0]
    FC = EMB // 128
    NT = HID // 128
    G = (B * N_ANTS) // GRP

    amen = np.ascontiguousarray(all_mentions.astype(bf))

    def wt_block(Wcols, scale=1.0, dtype=bf):
        # [1024, 1024] f32 block -> [128, FC, HID] (feature on partitions)
        wt = Wcols.T.reshape(FC, 128, HID).transpose(1, 0, 2) * scale
        if dtype is not bf:
            wt = np.clip(wt, -240.0, 240.0)
        return np.ascontiguousarray(wt.astype(dtype))

    S = FP8_SCALE if USE_FP8 else 1.0
    f8 = ml_dtypes.float8_e4m3
    wdt = f8 if USE_FP8 else bf
    w1at = wt_block(W1[:, 0:EMB], S)
    w1bt = wt_block(W1[:, EMB:2 * EMB], S, wdt)
    w1abt = wt_block(W1[:, 2 * EMB:3 * EMB], S, wdt)
    w1pw = np.zeros((128, HID), dtype=bf)
    w1pw[:PW] = (W1[:, 3 * EMB:3 * EMB + PW].T * S).astype(bf)
    w1pw[72] = (b1 * S).astype(bf)
    woutt = np.ascontiguousarray(Wout[0].reshape(NT, 128).T.astype(bf))

    in_maps = []
    for c in range(n_cores):
        rows = slice(c * B, (c + 1) * B)
        m_c = np.asarray(mentions_batch[rows], dtype=np.float32)       # [B, 1024]
        ment = np.ascontiguousarray(
            m_c.T.reshape(FC, 128, B).transpose(1, 0, 2).astype(bf))   # [128, FC, B]
        pw_c = np.asarray(pw_batch[rows], dtype=np.float32)            # [B, 64, 64]
        pwt = np.zeros((128, B * N_ANTS), dtype=bf)
        pwt[:PW] = pw_c.reshape(B * N_ANTS, PW).T.astype(bf)
        cols = np.arange(B * N_ANTS)
        for q in range(ROWS_PER_GRP):
            pwt[PW + q] = ((cols % GRP) // N_ANTS == q).astype(bf)
        pwt[72] = np.ones(B * N_ANTS, dtype=bf)
        idx_c = np.asarray(top_indices_batch[rows]).astype(np.int64).reshape(-1)
        idx_tiles = []
        for g in range(G):
            v = idx_c[g * GRP:(g + 1) * GRP].astype(np.int16)
            idx_tiles.append(np.tile(v.reshape(GRP // 16, 16).T, (8, 1)))
        idx = np.ascontiguousarray(np.concatenate(idx_tiles, axis=1))  # [128, G*32]
        rough = np.ascontiguousarray(
            np.asarray(top_rough_scores_batch[rows], dtype=np.float32).reshape(1, -1)
            + np.float32(np.asarray(bout).reshape(-1)[0]))
        in_maps.append({
            "amen": amen, "ment": ment, "w1bt": w1bt, "w1abt": w1abt,
            "w1at": w1at, "w1pw": w1pw, "woutt": woutt,
            "pwt": pwt, "idx": idx, "rough": rough,
        })
    return in_maps, B, n_tab


_NC_CACHE = {}


def kernel_with_results(all_mentions, mentions_batch, pw_batch, top_indices_batch,
                        top_rough_scores_batch, W1, b1, Wout, bout, **run_kwargs):
    args = [np.asarray(x) for x in (
        all_mentions, mentions_batch, pw_batch, top_indices_batch,
        top_rough_scores_batch, W1, b1, Wout, bout)]
    in_maps, B, n_tab = prep_inputs(*args)
    assert n_tab < 32768, "gather indices are int16"
    key = (B, n_tab)
    if key not in _NC_CACHE:
        _NC_CACHE[key] = build_nc(B, n_tab)
    nc = _NC_CACHE[key]
    res = None
    for attempt in range(3):
        try:
            res = run_bass_kernel_spmd(nc, in_maps, list(range(N_CORES)), **run_kwargs)
            break
        except Exception:
            if attempt == 2:
                raise
            import time
            time.sleep(5)
    scores = np.concatenate([np.asarray(r["out"]) for r in res.results], axis=0)
    batch = scores.shape[0]
    full = np.empty((batch, N_ANTS + 1), np.float32)
    full[:, 0] = EPS
    full[:, 1:] = scores
    return full, res


def kernel(**inputs) -> np.ndarray:
    out, _ = kernel_with_results(**inputs)
    return out


# revision 19
# speedup vs baseline: 1.2448x; 1.0044x over previous
"""Trainium2 Bass kernel for the AnaphoricityScorer (coref pairwise FFNN scorer).

Math (per batch row i, antecedent slot t):
    b  = all_mentions[top_indices[i, t]]                    # gathered mention
    pair = [a_i, b, a_i * b, pw[i, t]]                      # 3*1024 + 64 features
    h  = leaky_relu(pair @ W1.T + b1, 0.01)                 # 1024 hidden
    ffnn = h @ Wout.T + bout                                # scalar
    score = rough[i, t] + ffnn
    out = concat([eps_col, scores], axis=1)                 # [batch, 65]

Distribution: pure data parallel over the batch dim across 8 NeuronCores
(no collectives). all_mentions and FFNN weights are replicated.

Per-core algorithm (B = 128 batch rows -> 8192 pair rows, groups of 512):
  - b rows arrive transposed (features on partitions) straight from HBM via
    dma_gather(transpose=True), which is exactly the matmul rhs layout.
  - a*b is built by a DVE multiply against a stride-0 broadcast of mentions^T,
    written directly as fp8; b is cast bf16->fp8 on ScalarE.
  - The W1b / W1ab blocks run as fp8-e4m3 DoubleRow matmuls (two 128-feature
    k-tiles per instruction, 2 MACs/cell/cycle): 4 + 4 matmuls per
    (row-group, hidden-tile) instead of 16 bf16 ones. Weights are pre-scaled
    by FP8_SCALE on the host so 0.02-magnitude values clear fp8 denormals;
    the descale rides the Lrelu eviction's `scale` for free.
  - The a-term (a_i @ W1a.T, shared by all 64 antecedents of batch row i) and
    b1 are folded into the 9th (pw) matmul: its K=128 stationary tile carries
    W1pw in rows 0..63, the 8 per-group ha rows in 64..71 and b1 in row 72,
    while the static moving operand has matching one-hot / all-ones rows.
  - Lrelu on ScalarE evicts PSUM -> SBUF bf16 (applying 1/FP8_SCALE); the
    Wout reduction runs as col-tiled M=1 matmuls, 4 packed per PE pass via
    tile_position, deferred so they never stall the main pipeline.
  - Emission is software-pipelined one group ahead (gather + multiplies for
    group g+1 precede group g's matmuls) with deep tile pools so the PE
    stream never waits on SWDGE gathers.
"""

import sys

for _p in ("/opt/trn_rl_repo",):
    if _p not in sys.path:
        sys.path.append(_p)

import numpy as np
import ml_dtypes

import concourse.bacc as bacc
import concourse.mybir as mybir
from concourse.tile import TileContext
from concourse.bass_utils import run_bass_kernel_spmd

BF16 = mybir.dt.bfloat16
F32 = mybir.dt.float32
I16 = mybir.dt.int16
FP8 = mybir.dt.float8e4

USE_FP8 = True       # b/ab blocks in fp8-e4m3 DoubleRow (2 k-tiles per matmul)
FP8_SCALE = 512.0    # weight pre-scale so 0.02-magnitude weights leave fp8 denormals

N_CORES = 8
EMB = 1024
HID = 1024
N_ANTS = 64
PW = 64
EPS = 1e-7
GRP = 512          # pair rows per group (= 8 batch rows)
ROWS_PER_GRP = 8   # batch rows per group


def build_nc(B: int, n_tab: int):
    """Build the per-core Bass graph. B = batch rows per core."""
    G = (B * N_ANTS) // GRP  # number of row groups
    FC = EMB // 128          # 8 feature k-tiles per 1024-feature block
    NT = HID // 128          # 8 hidden tiles

    nc = bacc.Bacc("TRN2")
    amen = nc.declare_dram_parameter("amen", [n_tab, EMB], BF16, isOutput=False)
    ment = nc.declare_dram_parameter("ment", [128, FC, B], BF16, isOutput=False)
    wdt = FP8 if USE_FP8 else BF16
    w1bt = nc.declare_dram_parameter("w1bt", [128, FC, HID], wdt, isOutput=False)
    w1abt = nc.declare_dram_parameter("w1abt", [128, FC, HID], wdt, isOutput=False)
    w1at = nc.declare_dram_parameter("w1at", [128, FC, HID], BF16, isOutput=False)
    w1pw = nc.declare_dram_parameter("w1pw", [128, HID], BF16, isOutput=False)
    woutt = nc.declare_dram_parameter("woutt", [128, NT], BF16, isOutput=False)
    pwt = nc.declare_dram_parameter("pwt", [128, B * N_ANTS], BF16, isOutput=False)
    idx = nc.declare_dram_parameter("idx", [128, G * (GRP // 16)], I16, isOutput=False)
    rough = nc.declare_dram_parameter("rough", [1, B * N_ANTS], F32, isOutput=False)
    out = nc.declare_dram_parameter("out", [B, N_ANTS], F32, isOutput=True)

    with TileContext(nc) as tc:
        with (
            tc.tile_pool(name="const", bufs=1) as const,
            tc.tile_pool(name="btp", bufs=5) as btp,
            tc.tile_pool(name="abtp", bufs=4) as abtp,
            tc.tile_pool(name="bt8p", bufs=4) as bt8p,
            tc.tile_pool(name="wgp", bufs=3) as wgp,
            tc.tile_pool(name="htp", bufs=10) as htp,
            tc.tile_pool(name="rpool", bufs=3) as rpool,
            tc.tile_pool(name="spool", bufs=2) as spool,
            tc.tile_pool(name="psum", bufs=4, space="PSUM") as psum_pool,
            tc.tile_pool(name="psum_s", bufs=2, space="PSUM") as psum_s_pool,
        ):
            # ---- resident loads (gather + prologue deps first) ------------
            idx_t = const.tile([128, G * (GRP // 16)], I16)
            nc.sync.dma_start(idx_t[:], idx[:, :])
            ment_t = const.tile([128, FC, B], BF16)
            nc.sync.dma_start(ment_t[:], ment[:, :, :])
            w1at_t = const.tile([128, FC, HID], BF16)
            nc.sync.dma_start(w1at_t[:], w1at[:, :, :])
            w1bt_t = const.tile([128, FC, HID], wdt)
            nc.sync.dma_start(w1bt_t[:], w1bt[:, :, :])
            w1abt_t = const.tile([128, FC, HID], wdt)
            nc.sync.dma_start(w1abt_t[:], w1abt[:, :, :])
            w1pw_t = const.tile([128, HID], BF16)
            nc.sync.dma_start(w1pw_t[:], w1pw[:, :])
            woutt_t = const.tile([128, NT], BF16)
            nc.sync.dma_start(woutt_t[:], woutt[:, :])
            pwt_t = const.tile([128, B * N_ANTS], BF16)
            nc.sync.dma_start(pwt_t[:], pwt[:, :])
            # ---- prologue: ha = mentions @ (W1a*S).T, rows-on-partitions --
            # ha2r regroups ha so group g's 8 batch rows sit on partitions
            # 64..71 of the per-group weight tile wg (spliced below); the
            # static pwt operand carries one-hot rows that select the batch
            # row, folding the a-term (and b1 via an all-ones row) into the
            # pw matmul for free.
            ha2 = const.tile([B, HID], BF16)
            for half in range(HID // 512):
                pp = psum_s_pool.tile([B, 512], F32)
                for fc in range(FC):
                    nc.tensor.matmul(
                        pp[:],
                        ment_t[:, fc, :],
                        w1at_t[:, fc, half * 512:(half + 1) * 512],
                        start=(fc == 0),
                        stop=(fc == FC - 1),
                    )
                nc.scalar.activation(
                    ha2[:, half * 512:(half + 1) * 512], pp[:],
                    mybir.ActivationFunctionType.Identity,
                )
            ha2_dram = nc.dram_tensor("ha2_scratch", [B, HID], BF16)
            nc.sync.dma_start(ha2_dram[:, :], ha2[:])
            ha2r = const.tile([8, G, HID], BF16)
            nc.sync.dma_start(
                ha2r[:],
                ha2_dram[:, :].rearrange("(g q) n -> q g n", q=ROWS_PER_GRP),
            )

            # HAM warm-up: keep the PE streaming (and the clock gate open)
            # while the first gathers + casts land; the result is never read.
            wps = psum_s_pool.tile([B, 512], F32, tag="pp")
            for w in range(24):
                fc = w % FC
                nc.tensor.matmul(
                    wps[:], ment_t[:, fc, :], w1at_t[:, fc, 0:512],
                    start=(w == 0), stop=(w == 23),
                )

            # ---- main loop over row groups --------------------------------
            # Software-pipelined emission: the gather + a*b multiplies for
            # group g+1 are emitted BEFORE group g's matmuls so the DVE
            # stream reaches them early, and each (g, nt) second-matmul is
            # deferred by one nt so its ht dependency never stalls PE.
            def produce_group(g):
                r0 = g * ROWS_PER_GRP
                rtile = rpool.tile([1, GRP], F32)
                nc.sync.dma_start(rtile[:], rough[0:1, g * GRP:(g + 1) * GRP])
                bt = btp.tile([128, FC, GRP], BF16)
                nc.gpsimd.dma_gather(
                    bt[:], amen[:, :],
                    idx_t[:, g * (GRP // 16):(g + 1) * (GRP // 16)],
                    GRP, GRP, EMB, transpose=True,
                )
                abt = abtp.tile([128, FC, GRP], FP8 if USE_FP8 else BF16)
                a_b = ment_t[:, :, r0:r0 + ROWS_PER_GRP]
                for fc in range(FC):
                    nc.vector.tensor_mul(
                        abt[:, fc, :].rearrange("p (a b) -> p a b", a=ROWS_PER_GRP),
                        bt[:, fc, :].rearrange("p (a b) -> p a b", a=ROWS_PER_GRP),
                        a_b[:, fc, :].unsqueeze(2).to_broadcast(
                            [128, ROWS_PER_GRP, N_ANTS]),
                    )
                if USE_FP8:
                    bt8 = bt8p.tile([128, FC, GRP], FP8)
                    for fc in range(FC):
                        nc.scalar.activation(
                            bt8[:, fc, :], bt[:, fc, :],
                            mybir.ActivationFunctionType.Identity)
                    bt = bt8
                wg = wgp.tile([128, HID], BF16)
                nc.vector.tensor_copy(wg[:], w1pw_t[:])
                nc.vector.tensor_copy(wg[64:72, :], ha2r[:, g, :])
                return bt, abt, rtile, wg

            def emit_batch(ps4, hts, nts, start):
                # 4 M=1 matmuls packed into distinct PE column groups -- they
                # execute concurrently in the array (one per 32-col strip)
                for nt_i, ht_i in zip(nts, hts):
                    j = nt_i % 4
                    nc.tensor.matmul(
                        ps4[32 * j:32 * j + 1, :], woutt_t[:, nt_i:nt_i + 1],
                        ht_i[:], tile_position=(0, 32 * j),
                        start=start, stop=not start,
                    )

            def finalize_group(ps4, p_g, p_rtile):
                # DVE may read at most one PSUM operand per op: chain the four
                # column-group partial rows through SBUF
                t1 = spool.tile([1, GRP], F32)
                nc.vector.tensor_add(t1[:], ps4[0:1, :], p_rtile[:])
                t2 = spool.tile([1, GRP], F32)
                nc.vector.tensor_add(t2[:], ps4[32:33, :], t1[:])
                t3 = spool.tile([1, GRP], F32)
                nc.vector.tensor_add(t3[:], ps4[64:65, :], t2[:])
                stile = spool.tile([1, GRP], F32)
                nc.vector.tensor_add(stile[:], ps4[96:97, :], t3[:])
                nc.sync.dma_start(
                    out[p_g * ROWS_PER_GRP:(p_g + 1) * ROWS_PER_GRP, :].unsqueeze(0),
                    stile[:].rearrange("p (r c) -> p r c", r=ROWS_PER_GRP),
                )

            tiles = {0: produce_group(0)}
            rtiles = {}
            prev_group = None  # (g, ps4, hts) awaiting its second batch
            for g in range(G):
                r0 = g * ROWS_PER_GRP
                bt, abt, rtiles[g], wg = tiles.pop(g)
                if g + 1 < G:
                    tiles[g + 1] = produce_group(g + 1)
                hts = []
                ps4 = None
                for nt in range(NT):
                    ps = psum_pool.tile([128, GRP], F32)
                    nsl = slice(nt * 128, (nt + 1) * 128)
                    if USE_FP8:
                        for fc in range(0, FC, 2):
                            nc.tensor.matmul(
                                ps[:], w1bt_t[:, fc:fc + 2, nsl], bt[:, fc:fc + 2, :],
                                perf_mode=mybir.MatmulPerfMode.DoubleRow,
                                start=(fc == 0), stop=False,
                            )
                        for fc in range(0, FC, 2):
                            nc.tensor.matmul(
                                ps[:], w1abt_t[:, fc:fc + 2, nsl], abt[:, fc:fc + 2, :],
                                perf_mode=mybir.MatmulPerfMode.DoubleRow,
                                start=False, stop=False,
                            )
                    else:
                        for fc in range(FC):
                            nc.tensor.matmul(
                                ps[:], w1bt_t[:, fc, nsl], bt[:, fc, :],
                                start=(fc == 0), stop=False,
                            )
                        for fc in range(FC):
                            nc.tensor.matmul(
                                ps[:], w1abt_t[:, fc, nsl], abt[:, fc, :],
                                start=False, stop=False,
                            )
                    nc.tensor.matmul(
                        ps[:], wg[:, nsl],
                        pwt_t[:, g * GRP:(g + 1) * GRP],
                        start=False, stop=True,
                    )
                    ht = htp.tile([128, GRP], BF16)
                    nc.scalar.activation(
                        ht[:], ps[:],
                        mybir.ActivationFunctionType.Lrelu, alpha=0.01,
                        scale=(1.0 / FP8_SCALE) if USE_FP8 else 1.0,
                    )
                    hts.append(ht)
                    if nt == 1 and prev_group is not None:
                        p_g, p_ps4, p_hts = prev_group
                        emit_batch(p_ps4, p_hts[4:8], range(4, 8), start=False)
                        finalize_group(p_ps4, p_g, rtiles.pop(p_g))
                        prev_group = None
                    if nt == 5:
                        ps4 = psum_s_pool.tile([128, GRP], F32)
                        emit_batch(ps4, hts[0:4], range(0, 4), start=True)
                prev_group = (g, ps4, hts)
            # flush the last group's second batch
            p_g, p_ps4, p_hts = prev_group
            emit_batch(p_ps4, p_hts[4:8], range(4, 8), start=False)
            finalize_group(p_ps4, p_g, rtiles.pop(p_g))

    nc.compile()
    return nc


def prep_inputs(all_mentions, mentions_batch, pw_batch, top_indices_batch,
                top_rough_scores_batch, W1, b1, Wout, bout, n_cores=N_CORES):
    """Host-side marshalling: shard over batch, cast/transpose into the
    layouts the kernel expects. Returns (in_maps, B, n_tab, bout_val)."""
    bf = ml_dtypes.bfloat16
    batch = mentions_batch.shape[0]
    B = batch // n_cores
    n_tab = all_mentions.shape[0]
    FC = EMB // 128
    NT = HID // 128
    G = (B * N_ANTS) // GRP

    amen = np.ascontiguousarray(all_mentions.astype(bf))

    def wt_block(Wcols, scale=1.0, dtype=bf):
        # [1024, 1024] f32 block -> [128, FC, HID] (feature on partitions)
        wt = Wcols.T.reshape(FC, 128, HID).transpose(1, 0, 2) * scale
        if dtype is not bf:
            wt = np.clip(wt, -240.0, 240.0)
        return np.ascontiguousarray(wt.astype(dtype))

    S = FP8_SCALE if USE_FP8 else 1.0
    f8 = ml_dtypes.float8_e4m3
    wdt = f8 if USE_FP8 else bf
    w1at = wt_block(W1[:, 0:EMB], S)
    w1bt = wt_block(W1[:, EMB:2 * EMB], S, wdt)
    w1abt = wt_block(W1[:, 2 * EMB:3 * EMB], S, wdt)
    w1pw = np.zeros((128, HID), dtype=bf)
    w1pw[:PW] = (W1[:, 3 * EMB:3 * EMB + PW].T * S).astype(bf)
    w1pw[72] = (b1 * S).astype(bf)
    woutt = np.ascontiguousarray(Wout[0].reshape(NT, 128).T.astype(bf))

    in_maps = []
    for c in range(n_cores):
        rows = slice(c * B, (c + 1) * B)
        m_c = np.asarray(mentions_batch[rows], dtype=np.float32)       # [B, 1024]
        ment = np.ascontiguousarray(
            m_c.T.reshape(FC, 128, B).transpose(1, 0, 2).astype(bf))   # [128, FC, B]
        pw_c = np.asarray(pw_batch[rows], dtype=np.float32)            # [B, 64, 64]
        pwt = np.zeros((128, B * N_ANTS), dtype=bf)
        pwt[:PW] = pw_c.reshape(B * N_ANTS, PW).T.astype(bf)
        cols = np.arange(B * N_ANTS)
        for q in range(ROWS_PER_GRP):
            pwt[PW + q] = ((cols % GRP) // N_ANTS == q).astype(bf)
        pwt[72] = np.ones(B * N_ANTS, dtype=bf)
        idx_c = np.asarray(top_indices_batch[rows]).astype(np.int64).reshape(-1)
        idx_tiles = []
        for g in range(G):
            v = idx_c[g * GRP:(g + 1) * GRP].astype(np.int16)
            idx_tiles.append(np.tile(v.reshape(GRP // 16, 16).T, (8, 1)))
        idx = np.ascontiguousarray(np.concatenate(idx_tiles, axis=1))  # [128, G*32]
        rough = np.ascontiguousarray(
            np.asarray(top_rough_scores_batch[rows], dtype=np.float32).reshape(1, -1)
            + np.float32(np.asarray(bout).reshape(-1)[0]))
        in_maps.append({
            "amen": amen, "ment": ment, "w1bt": w1bt, "w1abt": w1abt,
            "w1at": w1at, "w1pw": w1pw, "woutt": woutt,
            "pwt": pwt, "idx": idx, "rough": rough,
        })
    return in_maps, B, n_tab


_NC_CACHE = {}


def kernel_with_results(all_mentions, mentions_batch, pw_batch, top_indices_batch,
                        top_rough_scores_batch, W1, b1, Wout, bout, **run_kwargs):
    args = [np.asarray(x) for x in (
        all_mentions, mentions_batch, pw_batch, top_indices_batch,
        top_rough_scores_batch, W1, b1, Wout, bout)]
    in_maps, B, n_tab = prep_inputs(*args)
    assert n_tab < 32768, "gather indices are int16"
    key = (B, n_tab)
    if key not in _NC_CACHE:
        _NC_CACHE[key] = build_nc(B, n_tab)
    nc = _NC_CACHE[key]
    res = None
    for attempt in range(3):
        try:
            res = run_bass_kernel_spmd(nc, in_maps, list(range(N_CORES)), **run_kwargs)
            break
        except Exception:
            if attempt == 2:
                raise
            import time
            time.sleep(5)
    scores = np.concatenate([np.asarray(r["out"]) for r in res.results], axis=0)
    batch = scores.shape[0]
    full = np.empty((batch, N_ANTS + 1), np.float32)
    full[:, 0] = EPS
    full[:, 1:] = scores
    return full, res


def kernel(**inputs) -> np.ndarray:
    out, _ = kernel_with_results(**inputs)
    return out
